# revision 1
# baseline (speedup 1.0000x reference)
"""CRF Viterbi decode (nn_CRF, B=512 T=512 O=64) on 8 Trainium2 NeuronCores.

Pure data parallel: the batch is sharded 64 sequences per core; the tiny
(64, 64) transition matrix and derived constants are replicated.

Per-core layout: g = j_hi in {0,1}; partition p = g*64 + b; tag j = g*32+j_lo.

Forward (per step t; state freezing is unnecessary because the backward
pass resets at t == L-1):
  ts[p, j_lo, i] = trans_rep[p, j_lo, i] + state2[p, i]  (DVE TT; the state
                   is read straight from PSUM via a stride-0 broadcast AP)
  m2[p, j_lo]    = max_i ts                              (DVE segmented reduce)
  hist[:, t, :]  = m2 + x2[:, t, :]                      (DVE TT; this IS the
                   state history, stored in split layout)
  psum_state[:, h*32:(h+1)*32] = S_h.T @ hist[:, t, :]   (PE "fold" with 0/1
                   selection matrices; PE is the only engine that can cross
                   partitions cheaply, rebuilding the replicated layout)

Backward, aligned in time t = T-1..0 (no reverse_sequence anywhere):
maintains the one-hot h of tag_{t+1}. Each iteration assembles
  cand[b, :] = trans[:, tag_{t+1}] + state_t[b, :]
entirely in PSUM with accumulating matmuls: a K=1 zeroing matmul, two
K=128 history-fold matmuls (column-sliced S0/S1 selectors), and four
K=32 block matmuls of the DVE-block-transposed one-hot hBT against
straight/cross copies of trans.T (the 32x32 DVE transpose only permutes
within blocks; the cross table fixes up the off-diagonal blocks).
Exact first-argmax (ties broken like jnp.argmax, fp32-exact):
  negmax = -max(cand); h_any = (cand + negmax == 0);
  t1 = (h_any * ne) * (64 - i); mi = -max(t1) = i* - 64;
  h = ((64 - i) + mi == 0).
The sequence-end reset rides on ne = not_end[:, t-1]: ne=0 poisons mi to 0,
making h all-zero, which zeroes the next transition gather so cand
collapses to hist[:, t, :] — reproducing init_tag/init_conf of the
reference exactly. Tag numbers are recovered for free from mi (tag =
mi + 64, fixed up in the bulk epilogue). Confidence = 1/sum exp(cand -
max) via the ACT engine's Exp with per-partition bias and accumulator.

Positions >= L are zeroed by the mask, matching the reference.

Hardware caveats encoded here (cost several debugging hours):
- matmul operands at partition base 64 crash the device (PE quadrant-3
  bug) — all contractions stay at base 0/32;
- start_tensor_calc=True lazily zeroes the whole per-partition 2KB PSUM
  region, so each accumulation group has exactly one start=True (the
  zeroing matmul) and everything else accumulates.
"""
import numpy as np

_B, _T, _O = 512, 512, 64
_NCORES = 8
_BL = _B // _NCORES

_CACHE = {}

_WORK_BUFS = 2
_PST_BUFS = 3
_PBW_BUFS = 3


def _host_constants(trans):
    trans = np.ascontiguousarray(trans.astype(np.float32))
    transT = np.ascontiguousarray(trans.T)                  # [j, i]
    tr = transT.reshape(2, 32, 64)
    trans_rep = np.ascontiguousarray(
        np.broadcast_to(tr[:, None, :, :], (2, 64, 32, 64)).reshape(128, 32, 64)
    )
    S = np.zeros((2, 128, 128), np.float32)
    for h in range(2):
        for b in range(64):
            S[h, h * 64 + b, b] = 1.0
            S[h, h * 64 + b, 64 + b] = 1.0
    tio_s = np.ascontiguousarray(transT)                    # [64, 64]
    tio_c = np.ascontiguousarray(
        np.concatenate([tio_s[32:64], tio_s[0:32]], axis=0))
    ipair = np.ascontiguousarray(
        np.concatenate([np.eye(64, dtype=np.float32),
                        np.eye(64, dtype=np.float32)], axis=0))
    bmi = np.ascontiguousarray(
        np.broadcast_to(64.0 - np.arange(64, dtype=np.float32), (64, 64)))
    return {
        "trans_rep": trans_rep,
        "S0": np.ascontiguousarray(S[0]),
        "S1": np.ascontiguousarray(S[1]),
        "tio_s": tio_s,
        "tio_c": tio_c,
        "ipair": ipair,
        "bmi": bmi,
    }


def _host_percore(logits_c, seq_c, T):
    x2 = np.ascontiguousarray(
        logits_c.astype(np.float32)
        .reshape(_BL, T, 2, 32).transpose(2, 0, 1, 3).reshape(128, T, 32)
    )
    not_end = np.ones((_BL, T), np.float32)
    not_end[np.arange(_BL), np.maximum(seq_c - 1, 0)] = 0.0
    mask = (np.arange(T)[None, :] < seq_c[:, None]).astype(np.float32)
    return {"x2": x2, "not_end": not_end, "mask": mask}


def _build_tile_program(tc, outs, ins, T, CT=64):
    from contextlib import ExitStack
    import concourse.bass as bass
    from concourse import mybir
    from concourse.tile import add_dep_helper

    F32 = mybir.dt.float32
    AX = mybir.AxisListType
    OP = mybir.AluOpType
    ACT = mybir.ActivationFunctionType

    nc = tc.nc
    tags_d, conf_d = outs
    (x2_d, notend_d, mask_d, transrep_d, s0_d, s1_d, tios_d, tioc_d,
     ipair_d, bmi_d) = ins

    def bcast_mid(ap2d, n):
        assert len(ap2d.ap) == 2, ap2d.ap
        return bass.AP(tensor=ap2d.tensor, offset=ap2d.offset,
                       ap=[ap2d.ap[0], [0, n], ap2d.ap[1]])

    with ExitStack() as ctx:
        consts = ctx.enter_context(tc.tile_pool(name="consts", bufs=1))
        big = ctx.enter_context(tc.tile_pool(name="big", bufs=1))
        work = ctx.enter_context(tc.tile_pool(name="work", bufs=_WORK_BUFS))
        tspool = ctx.enter_context(tc.tile_pool(name="tspool", bufs=2))
        xchunks = ctx.enter_context(tc.tile_pool(name="xchunks", bufs=3))
        pst = ctx.enter_context(
            tc.tile_pool(name="pstate", bufs=_PST_BUFS, space="PSUM"))
        pbw = ctx.enter_context(
            tc.tile_pool(name="pbw", bufs=_PBW_BUFS, space="PSUM"))

        trans_rep = consts.tile([128, 32, 64], F32)
        nc.sync.dma_start(trans_rep, transrep_d)
        S0 = consts.tile([128, 128], F32)
        nc.sync.dma_start(S0, s0_d)
        S1 = consts.tile([128, 128], F32)
        nc.sync.dma_start(S1, s1_d)
        tio_s = consts.tile([64, 64], F32)
        nc.sync.dma_start(tio_s, tios_d)
        tio_c = consts.tile([64, 64], F32)
        nc.sync.dma_start(tio_c, tioc_d)
        ipair = consts.tile([128, 64], F32)
        nc.sync.dma_start(ipair, ipair_d)
        bmi = consts.tile([64, 64], F32)
        nc.sync.dma_start(bmi, bmi_d)
        notend = consts.tile([64, T], F32)
        nc.sync.dma_start(notend, notend_d)
        maskt = consts.tile([64, T], F32)
        nc.sync.dma_start(maskt, mask_d)

        hist = big.tile([128, T, 32], F32)
        scoreb = big.tile([64, T], F32)
        mib = big.tile([64, T], F32)
        zl = consts.tile([1, 64], F32)
        nc.vector.memset(zl, 0.0)
        zr = consts.tile([1, 64], F32)
        nc.vector.memset(zr, 0.0)

        # ---------------- forward ----------------
        nchunks = (T + CT - 1) // CT
        psum_state = None
        for c in range(nchunks):
            t0 = c * CT
            ct = min(CT, T - t0)
            xc = xchunks.tile([128, CT, 32], F32, tag="xc")
            nc.sync.dma_start(xc[:, :ct, :], x2_d[:, t0:t0 + ct, :])
            for tt in range(ct):
                t = t0 + tt
                if t == 0:
                    nc.scalar.copy(hist[:, 0, :], xc[:, 0, :])
                else:
                    ts_t = tspool.tile([128, 32, 64], F32, tag="ts")
                    nc.vector.tensor_tensor(
                        out=ts_t, in0=trans_rep,
                        in1=bcast_mid(psum_state[:], 32), op=OP.add)
                    m2 = work.tile([128, 32], F32, tag="m2")
                    nc.vector.tensor_reduce(m2, ts_t, axis=AX.X, op=OP.max)
                    nc.vector.tensor_tensor(out=hist[:, t, :], in0=m2,
                                            in1=xc[:, tt, :], op=OP.add)
                psum_state = pst.tile([128, 64], F32, tag="pstate")
                nc.tensor.matmul(psum_state[:, 0:32], S0, hist[:, t, :],
                                 start=True, stop=True)
                nc.tensor.matmul(psum_state[:, 32:64], S1, hist[:, t, :],
                                 start=True, stop=True)

        # ---------------- backward ----------------
        def chain_mms(insts):
            for a, b in zip(insts[1:], insts[:-1]):
                add_dep_helper(a.ins, b.ins, sync=False,
                               reason="psum accumulation order")
            return insts[-1]

        def hist_fold_mms(ps, t):
            # Full-tile start=True pair; no dep on the backward chain, so
            # these pre-run and stay off the critical path. K=128 with
            # column-sliced S0/S1 selectors: operands at partition base 64
            # hit a PE quadrant-3 HW bug, so everything stays at base 0.
            # Exactly ONE start=True per psum tile (a K=1 zeroing matmul):
            # on HW, start_tensor_calc lazily zeroes the whole per-partition
            # 2KB region, so a second start=True would wipe earlier columns
            # for accumulation readers. Everything else accumulates.
            i0 = nc.tensor.matmul(ps[:, :], zl, zr,
                                  start=True, stop=False,
                                  skip_group_check=True)
            i1 = nc.tensor.matmul(ps[:, 0:32], S0[:, 0:64], hist[:, t, :],
                                  start=False, stop=False,
                                  skip_group_check=True)
            i2 = nc.tensor.matmul(ps[:, 32:64], S1[:, 0:64], hist[:, t, :],
                                  start=False, stop=True,
                                  skip_group_check=True)
            return chain_mms([i0, i1, i2])

        def h_mms(ps, hBT, after):
            i1 = nc.tensor.matmul(ps[0:32, :], hBT[0:32, 0:32],
                                  tio_s[0:32, :], start=False, stop=False,
                                  skip_group_check=True)
            i2 = nc.tensor.matmul(ps[0:32, :], hBT[0:32, 32:64],
                                  tio_c[0:32, :], start=False, stop=False,
                                  skip_group_check=True)
            i3 = nc.tensor.matmul(ps[32:64, :], hBT[32:64, 0:32],
                                  tio_c[32:64, :], start=False, stop=False,
                                  skip_group_check=True)
            i4 = nc.tensor.matmul(ps[32:64, :], hBT[32:64, 32:64],
                                  tio_s[32:64, :], start=False, stop=True,
                                  skip_group_check=True)
            return chain_mms([after, i1, i2, i3, i4])

        def bwd_dve(cand_ap, t, ne_scalar):
            negmax = work.tile([64, 1], F32, tag="negmax")
            nc.vector.tensor_reduce(negmax, cand_ap, axis=AX.X, op=OP.max,
                                    negate=True)
            h_any = work.tile([64, 64], F32, tag="h_any")
            nc.vector.tensor_scalar(out=h_any, in0=cand_ap, scalar1=negmax,
                                    scalar2=0.0, op0=OP.add, op1=OP.is_equal)
            t1 = work.tile([64, 64], F32, tag="t1")
            nc.vector.scalar_tensor_tensor(out=t1, in0=h_any,
                                           scalar=ne_scalar, in1=bmi,
                                           op0=OP.mult, op1=OP.mult)
            mi = mib[:, t:t + 1]
            nc.vector.tensor_reduce(mi, t1, axis=AX.X, op=OP.max, negate=True)
            h = work.tile([64, 64], F32, tag="h")
            nc.vector.tensor_scalar(out=h, in0=bmi, scalar1=mi,
                                    scalar2=0.0, op0=OP.add, op1=OP.is_equal)
            hBT = work.tile([64, 64], F32, tag="hBT")
            nc.vector.transpose(hBT, h)
            e = work.tile([64, 64], F32, tag="e")
            nc.scalar.activation(out=e, in_=cand_ap, func=ACT.Exp,
                                 bias=negmax, scale=1.0,
                                 accum_out=scoreb[:, t:t + 1])
            return hBT

        init_ps = pbw.tile([64, 64], F32, tag="bwps")
        hist_fold_mms(init_ps, T - 1)
        hBT = bwd_dve(init_ps[:], T - 1, notend[:, T - 2:T - 1])

        for t in range(T - 2, -1, -1):
            ps = pbw.tile([64, 64], F32, tag="bwps")
            last = hist_fold_mms(ps, t)
            h_mms(ps, hBT, after=last)
            ne = notend[:, t - 1:t] if t >= 1 else 1.0
            hBT = bwd_dve(ps[:], t, ne)

        # ---------------- epilogue ----------------
        recip = work.tile([64, T], F32, tag="recip")
        nc.vector.reciprocal(recip, scoreb)
        conf = work.tile([64, T], F32, tag="conf")
        nc.vector.tensor_tensor(out=conf, in0=recip, in1=maskt, op=OP.mult)
        nc.sync.dma_start(conf_d, conf)
        tagsm = work.tile([64, T], F32, tag="tagsm")
        nc.vector.scalar_tensor_tensor(out=tagsm, in0=mib, scalar=64.0,
                                       in1=maskt, op0=OP.add, op1=OP.mult)
        tagsi = work.tile([64, T], mybir.dt.int32, tag="tagsi")
        nc.vector.tensor_copy(tagsi, tagsm)
        nc.sync.dma_start(tags_d, tagsi)


def _get_compiled(T):
    key = ("nc", T)
    if key in _CACHE:
        return _CACHE[key]
    import concourse.bacc as bacc
    import concourse.tile as tile
    from concourse import mybir

    F32 = mybir.dt.float32
    I32 = mybir.dt.int32
    nc = bacc.Bacc("TRN2", target_bir_lowering=False, debug=False,
                   num_devices=_NCORES)

    ins_spec = [
        ("x2", [128, T, 32], F32),
        ("not_end", [64, T], F32),
        ("mask", [64, T], F32),
        ("trans_rep", [128, 32, 64], F32),
        ("S0", [128, 128], F32),
        ("S1", [128, 128], F32),
        ("tio_s", [64, 64], F32),
        ("tio_c", [64, 64], F32),
        ("ipair", [128, 64], F32),
        ("bmi", [64, 64], F32),
    ]
    ins = tuple(
        nc.dram_tensor(name, shape, dt, kind="ExternalInput").ap()
        for name, shape, dt in ins_spec
    )
    outs = (
        nc.dram_tensor("tags", [64, T], I32, kind="ExternalOutput").ap(),
        nc.dram_tensor("conf", [64, T], F32, kind="ExternalOutput").ap(),
    )

    with tile.TileContext(nc) as tc:
        _build_tile_program(tc, outs, ins, T=T)
    nc.compile()
    _CACHE[key] = nc
    return nc


def _run(logits, transition_params, sequence_lengths, trace=False):
    from concourse.bass_utils import run_bass_kernel_spmd

    T = logits.shape[1]
    logits = np.asarray(logits, dtype=np.float32)
    trans = np.asarray(transition_params, dtype=np.float32)
    seq = np.asarray(sequence_lengths, dtype=np.int32)

    consts = _host_constants(trans)
    in_maps = []
    for c in range(_NCORES):
        sl = slice(c * _BL, (c + 1) * _BL)
        pc = _host_percore(logits[sl], seq[sl], T)
        m = {"x2": pc["x2"], "not_end": pc["not_end"], "mask": pc["mask"]}
        m.update(consts)
        in_maps.append(m)

    nc = _get_compiled(T)
    res = run_bass_kernel_spmd(nc, in_maps, list(range(_NCORES)),
                               trace=trace)
    tags = np.concatenate([np.asarray(res.results[c]["tags"])
                           for c in range(_NCORES)], axis=0)
    conf = np.concatenate([np.asarray(res.results[c]["conf"])
                           for c in range(_NCORES)], axis=0)
    return (tags.astype(np.int32), conf.astype(np.float32)), res


def kernel(logits, transition_params, sequence_lengths):
    (tags, conf), _ = _run(logits, transition_params, sequence_lengths)
    return tags, conf



# revision 3
# speedup vs baseline: 1.2656x; 1.2656x over previous
"""CRF Viterbi decode (nn_CRF, B=512 T=512 O=64) on 8 Trainium2 NeuronCores.

Pure data parallel: 64 sequences per core; the (64, 64) transition matrix and
derived constants are replicated.

Per-core layout: g = j_hi in {0,1}; partition p = g*64 + b; tag j = g*32+j_lo.

Forward (per step t), DVE ~96% utilized with Pool assisting:
  DVE:  hist = x2[:, t-1, :] + m2buf[:, t-1, :]  (the split-layout state),
        hX = cross-half partition swap of hist via TWO stream_shuffle ops
        with offset partition bases (verified on HW: the shuffle crossbar
        honors cross-quadrant in/out AP bases),
        tsA = trans_rep[:, 0:CA, :] + psum_state bcast  (columns 0..CA-1),
        BOTH segmented max-reduces -> m2buf[:, t, :]  (only DVE can
        max-reduce: GPSIMD's software TensorTensor implements add/mult only
        and cannot touch PSUM).
  Pool: adds for columns CA..31 in per-partition [own-half, cross-half]
        i-order (max is order-invariant) reading hist/hX straight from
        SBUF -- no PE fold or ACT copy on its path.
  PE:   psum_state(t) = S@x2[:, t, :] + S@m2buf[:, t, :]; the x folds are
        PSUM-accumulated so hist is never needed in replicated layout;
        fl(x + m2) in PSUM matches the reference's fp32 add exactly.

Backward, aligned in time t = T-1..0: cand_t = hist_t + trans[:, tag_{t+1}]
is built in PSUM from x2/m2 fold matmuls (pre-run, off the critical chain),
a Pool-computed hist window, and 4 one-hot h matmuls.  The argmax chain is
  max8 (top-8 values) -> max_index (first-index ties, exactly jnp.argmax,
  including the frequent exact fp32 ties at |cand|~1e3)
  idxf = fp32(idx); h = (iota == idxf) * ne   (ne=0 at t==L resets the
        chain so cand collapses to hist, reproducing init_tag/init_conf)
  hBT = 32x32-blockwise DVE transpose, fixed up by straight/cross tables in
        the 4 K=32 h matmuls (PE quadrant layout as in the proven baseline).
max_index writes its 8 indices straight into tags8[:, t, :]; tags come from
tags8[:, :, 0] in the bulk epilogue.  Confidence = 1/sum exp(cand - max) via
ACT Exp with per-partition bias (bias = -max via a tiny tensor_scalar).

Hardware caveats (cost several debugging sessions):
- GPSIMD cannot access PSUM, and its TensorTensor ucode has no max op;
- concurrent DVE+ACT (or DVE+anything) reads of one PSUM bank serialize on
  the bank read port -- keep per-engine state copies in separate banks or
  feed engines from SBUF;
- matmul operands at partition base 64 crash the device (PE quadrant-3 bug)
  -- all contractions stay at base 0/32;
- start_tensor_calc=True lazily zeroes the whole per-partition 2KB PSUM
  region, so each accumulation group has exactly one start=True (a K=1
  zeroing matmul) and everything else accumulates;
- hist_0 is x2[:, 0, :] alone (m2 exists only for t >= 1).
"""
import numpy as np

_B, _T, _O = 512, 512, 64
_NCORES = 8
_BL = _B // _NCORES

_CA = 21          # DVE-adds tag columns (j_lo 0.._CA-1); Pool adds the rest
_USE_MAXIDX = True  # max8/max_index argmax vs baseline 5-op argmax

_CACHE = {}


def _host_constants(trans):
    trans = np.ascontiguousarray(trans.astype(np.float32))
    transT = np.ascontiguousarray(trans.T)                  # [j, i]
    tr = transT.reshape(2, 32, 64)
    trans_rep = np.ascontiguousarray(
        np.broadcast_to(tr[:, None, :, :], (2, 64, 32, 64)).reshape(128, 32, 64)
    )
    S = np.zeros((2, 128, 128), np.float32)
    for h in range(2):
        for b in range(64):
            S[h, h * 64 + b, b] = 1.0
            S[h, h * 64 + b, 64 + b] = 1.0
    tio_s = np.ascontiguousarray(transT)                    # [64, 64]
    tio_c = np.ascontiguousarray(
        np.concatenate([tio_s[32:64], tio_s[0:32]], axis=0))
    iota = np.ascontiguousarray(
        np.broadcast_to(np.arange(64, dtype=np.float32), (64, 64)))
    # Pool B-column tables with per-partition [own-half, cross-half] i order:
    # trbB_own[p=(g,b), j_lo, i_lo] = trans[g*32+i_lo, g*32+(CA+j_lo)]
    # trbB_cross[p, j_lo, i_lo]     = trans[(1-g)*32+i_lo, g*32+(CA+j_lo)]
    CA, CB = _CA, 32 - _CA
    g = (np.arange(128) // 64)[:, None, None]           # [128,1,1]
    jl = (CA + np.arange(CB))[None, :, None]            # [1,CB,1]
    il = np.arange(32)[None, None, :]                   # [1,1,32]
    trbB_own = np.ascontiguousarray(
        trans[g * 32 + il, g * 32 + jl].astype(np.float32))
    trbB_cross = np.ascontiguousarray(
        trans[(1 - g) * 32 + il, g * 32 + jl].astype(np.float32))
    return {
        "trans_rep": trans_rep,
        "S0": np.ascontiguousarray(S[0]),
        "S1": np.ascontiguousarray(S[1]),
        "tio_s": tio_s,
        "tio_c": tio_c,
        "iota": iota,
        "trbB_own": trbB_own,
        "trbB_cross": trbB_cross,
    }


def _host_percore(logits_c, seq_c, T):
    x2 = np.ascontiguousarray(
        logits_c.astype(np.float32)
        .reshape(_BL, T, 2, 32).transpose(2, 0, 1, 3).reshape(128, T, 32)
    )
    # ne2[b, t] = 0 iff t == L_b: at backward step t == L the one-hot is
    # zeroed so cand_{L-1} collapses to hist_{L-1} (the reference's frozen
    # last_score).
    ne2 = np.ones((_BL, T), np.float32)
    sel = seq_c <= T - 1
    ne2[np.arange(_BL)[sel], seq_c[sel]] = 0.0
    mask = (np.arange(T)[None, :] < seq_c[:, None]).astype(np.float32)
    return {"x2": x2, "ne2": ne2, "mask": mask,
            "ne2u": ne2.astype(np.uint16)}


def _build_tile_program(tc, outs, ins, T):
    from contextlib import ExitStack
    import concourse.bass as bass
    from concourse import mybir
    from concourse.tile import add_dep_helper

    F32 = mybir.dt.float32
    U16 = mybir.dt.uint16
    AX = mybir.AxisListType
    OP = mybir.AluOpType
    ACT = mybir.ActivationFunctionType

    nc = tc.nc
    tags_d, conf_d = outs
    (x2_d, ne2_d, mask_d, transrep_d, s0_d, s1_d, tios_d, tioc_d,
     iota_d, trbo_d, trbc_d) = ins

    CA = _CA
    CB = 32 - CA

    def bcast_mid(ap2d, n):
        assert len(ap2d.ap) == 2, ap2d.ap
        return bass.AP(tensor=ap2d.tensor, offset=ap2d.offset,
                       ap=[ap2d.ap[0], [0, n], ap2d.ap[1]])

    def chain_mms(insts):
        for a, b in zip(insts[1:], insts[:-1]):
            add_dep_helper(a.ins, b.ins, sync=False,
                           reason="psum accumulation order")
        return insts[-1]

    with ExitStack() as ctx:
        consts = ctx.enter_context(tc.tile_pool(name="consts", bufs=1))
        big = ctx.enter_context(tc.tile_pool(name="big", bufs=1))
        work = ctx.enter_context(tc.tile_pool(name="work", bufs=2))
        tsA_p = ctx.enter_context(tc.tile_pool(name="tsA", bufs=2))
        tsB_p = ctx.enter_context(tc.tile_pool(name="tsB", bufs=2))
        histw = ctx.enter_context(tc.tile_pool(name="histw", bufs=4))
        pstA = ctx.enter_context(
            tc.tile_pool(name="pstA", bufs=3, space="PSUM"))
        pbw = ctx.enter_context(
            tc.tile_pool(name="pbw", bufs=3, space="PSUM"))
        hxp = ctx.enter_context(tc.tile_pool(name="hxp", bufs=2))

        trans_rep = consts.tile([128, 32, 64], F32)
        nc.sync.dma_start(trans_rep, transrep_d)
        S0 = consts.tile([128, 128], F32)
        nc.sync.dma_start(S0, s0_d)
        S1 = consts.tile([128, 128], F32)
        nc.sync.dma_start(S1, s1_d)
        tio_s = consts.tile([64, 64], F32)
        nc.sync.dma_start(tio_s, tios_d)
        tio_c = consts.tile([64, 64], F32)
        nc.sync.dma_start(tio_c, tioc_d)
        iota = consts.tile([64, 64], F32)
        nc.sync.dma_start(iota, iota_d)
        trbB_own = consts.tile([128, CB, 32], F32)
        nc.sync.dma_start(trbB_own, trbo_d)
        trbB_cross = consts.tile([128, CB, 32], F32)
        nc.sync.dma_start(trbB_cross, trbc_d)
        ne2 = consts.tile([64, T], F32)
        nc.sync.dma_start(ne2, ne2_d)
        maskt = consts.tile([64, T], F32)
        nc.sync.dma_start(maskt, mask_d)
        zl = consts.tile([1, 64], F32)
        nc.vector.memset(zl, 0.0)
        zl128 = consts.tile([1, 128], F32)
        nc.vector.memset(zl128, 0.0)
        zr128 = consts.tile([1, 128], F32)
        nc.vector.memset(zr128, 0.0)
        zr = consts.tile([1, 64], F32)
        nc.vector.memset(zr, 0.0)

        x2 = big.tile([128, T, 32], F32)
        NCH = 8
        CT = T // NCH
        for c in range(NCH):
            nc.sync.dma_start(x2[:, c * CT:(c + 1) * CT, :],
                              x2_d[:, c * CT:(c + 1) * CT, :])
        m2buf = big.tile([128, T, 32], F32)
        tags8 = big.tile([64, T, 8], U16)
        scoreb = big.tile([64, T], F32)

        # ---------------- forward ----------------
        # Division of labor (hardware constraints: GPSIMD cannot access
        # PSUM, and its software TensorTensor only implements add/mult —
        # no max): DVE reads the PSUM state directly and handles the adds
        # for columns 0..CA-1 plus ALL segmented max-reduces; Pool adds
        # columns CA..31 from an SBUF state copy made by the otherwise-idle
        # ACT engine.
        def fwd_fold(ps, t):
            """psum_state(t) = S@x2[:, t, :] (+ S@m2buf[:, t, :] if t>0)."""
            i0 = nc.tensor.matmul(ps[:, :], zl128, zr, start=True,
                                  stop=False, skip_group_check=True)
            mms = [i0]
            mms.append(nc.tensor.matmul(ps[:, 0:32], S0, x2[:, t, :],
                                        start=False, stop=False,
                                        skip_group_check=True))
            mms.append(nc.tensor.matmul(ps[:, 32:64], S1, x2[:, t, :],
                                        start=False, stop=(t == 0),
                                        skip_group_check=True))
            if t > 0:
                mms.append(nc.tensor.matmul(
                    ps[:, 0:CA], S0, m2buf[:, t, 0:CA],
                    start=False, stop=False, skip_group_check=True))
                mms.append(nc.tensor.matmul(
                    ps[:, 32:32 + CA], S1, m2buf[:, t, 0:CA],
                    start=False, stop=False, skip_group_check=True))
                mms.append(nc.tensor.matmul(
                    ps[:, CA:32], S0, m2buf[:, t, CA:32],
                    start=False, stop=False, skip_group_check=True))
                mms.append(nc.tensor.matmul(
                    ps[:, 32 + CA:64], S1, m2buf[:, t, CA:32],
                    start=False, stop=True, skip_group_check=True))
            chain_mms(mms)
            return ps

        psA_prev = pstA.tile([128, 64], F32, tag="psA")
        fwd_fold(psA_prev, 0)

        ident = list(range(32))
        for t in range(1, T):
            # DVE: previous state in split layout (own half per partition)
            # plus its cross-half partition swap, feeding Pool from SBUF.
            if t == 1:
                hist = x2[:, 0, :]
            else:
                histt = hxp.tile([128, 32], F32, tag="hist")
                nc.vector.tensor_tensor(out=histt, in0=x2[:, t - 1, :],
                                        in1=m2buf[:, t - 1, :], op=OP.add)
                hist = histt[:]
            hX = hxp.tile([128, 32], F32, tag="hX")
            nc.vector.stream_shuffle(hX[0:64, :], hist[64:128, :], ident)
            nc.vector.stream_shuffle(hX[64:128, :], hist[0:64, :], ident)
            # Pool: adds for columns CA..31 in [own, cross] i-order
            tsB = tsB_p.tile([128, CB, 64], F32, tag="tsB")
            nc.gpsimd.tensor_tensor(
                out=tsB[:, :, 0:32], in0=trbB_own,
                in1=bass.AP(tensor=hist.tensor, offset=hist.offset,
                            ap=[hist.ap[0], [0, CB], hist.ap[1]]),
                op=OP.add)
            nc.gpsimd.tensor_tensor(
                out=tsB[:, :, 32:64], in0=trbB_cross,
                in1=bcast_mid(hX[:], CB), op=OP.add)
            # DVE: adds for columns 0..CA-1, then both segmented reduces
            tsA = tsA_p.tile([128, CA, 64], F32, tag="tsA")
            nc.vector.tensor_tensor(
                out=tsA, in0=trans_rep[:, 0:CA, :],
                in1=bcast_mid(psA_prev[:], CA), op=OP.add)
            nc.vector.tensor_reduce(m2buf[:, t, 0:CA], tsA,
                                    axis=AX.X, op=OP.max)
            nc.vector.tensor_reduce(m2buf[:, t, CA:32], tsB,
                                    axis=AX.X, op=OP.max)

            psA = pstA.tile([128, 64], F32, tag="psA")
            fwd_fold(psA, t)
            psA_prev = psA

        # ---------------- backward ----------------
        def bwd_prep(ps, t, hb):
            """cand base: ps = hist_t replicated over i-columns."""
            i0 = nc.tensor.matmul(ps[:, :], zl, zr, start=True, stop=False,
                                  skip_group_check=True)
            i1 = nc.tensor.matmul(ps[:, 0:32], S0[:, 0:64], hb,
                                  start=False, stop=False,
                                  skip_group_check=True)
            i2 = nc.tensor.matmul(ps[:, 32:64], S1[:, 0:64], hb,
                                  start=False, stop=(t == T - 1),
                                  skip_group_check=True)
            return chain_mms([i0, i1, i2])

        def h_mms(ps, hBT, after):
            i1 = nc.tensor.matmul(ps[0:32, :], hBT[0:32, 0:32],
                                  tio_s[0:32, :], start=False, stop=False,
                                  skip_group_check=True)
            i2 = nc.tensor.matmul(ps[0:32, :], hBT[0:32, 32:64],
                                  tio_c[0:32, :], start=False, stop=False,
                                  skip_group_check=True)
            i3 = nc.tensor.matmul(ps[32:64, :], hBT[32:64, 0:32],
                                  tio_c[32:64, :], start=False, stop=False,
                                  skip_group_check=True)
            i4 = nc.tensor.matmul(ps[32:64, :], hBT[32:64, 32:64],
                                  tio_s[32:64, :], start=False, stop=True,
                                  skip_group_check=True)
            return chain_mms([after, i1, i2, i3, i4])

        def hist_tile(t):
            if t == 0:
                return x2[:, 0, :]  # init state: m2 only exists for t >= 1
            hb = histw.tile([128, 32], F32, tag="histw")
            nc.gpsimd.tensor_tensor(out=hb, in0=x2[:, t, :],
                                    in1=m2buf[:, t, :], op=OP.add)
            return hb

        def bwd_dve(ps, t):
            m8 = work.tile([64, 8], F32, tag="m8")
            nc.vector.max(m8, ps[:])
            nc.vector.max_index(tags8[:, t, :], m8, ps[:])
            idxf = work.tile([64, 1], F32, tag="idxf")
            nc.vector.tensor_copy(idxf, tags8[:, t, 0:1])
            h = work.tile([64, 64], F32, tag="h")
            nc.vector.tensor_scalar(out=h, in0=iota,
                                    scalar1=idxf,
                                    scalar2=ne2[:, t:t + 1],
                                    op0=OP.is_equal, op1=OP.mult)
            hBT = work.tile([64, 64], F32, tag="hBT")
            nc.vector.transpose(hBT, h)
            negmax = work.tile([64, 1], F32, tag="negmax")
            nc.vector.tensor_scalar(out=negmax, in0=m8[:, 0:1],
                                    scalar1=-1.0, scalar2=None, op0=OP.mult)
            e = work.tile([64, 64], F32, tag="e")
            nc.scalar.activation(out=e, in_=ps[:], func=ACT.Exp,
                                 bias=negmax, scale=1.0,
                                 accum_out=scoreb[:, t:t + 1])
            return hBT

        ps = pbw.tile([64, 64], F32, tag="bwps")
        bwd_prep(ps, T - 1, hist_tile(T - 1))
        hBT = bwd_dve(ps, T - 1)

        for t in range(T - 2, -1, -1):
            ps = pbw.tile([64, 64], F32, tag="bwps")
            last = bwd_prep(ps, t, hist_tile(t))
            h_mms(ps, hBT, after=last)
            hBT = bwd_dve(ps, t)

        # ---------------- epilogue ----------------
        tagsf = work.tile([64, T], F32, tag="tagsf")
        t8v = bass.AP(tensor=tags8.tensor, offset=tags8.offset,
                      ap=[tags8.ap[0], [8, T]])
        nc.vector.tensor_copy(tagsf, t8v)
        tagsm = work.tile([64, T], F32, tag="tagsm")
        nc.vector.tensor_tensor(out=tagsm, in0=tagsf, in1=maskt, op=OP.mult)
        tagsi = work.tile([64, T], mybir.dt.int32, tag="tagsi")
        nc.vector.tensor_copy(tagsi, tagsm)
        nc.sync.dma_start(tags_d, tagsi)
        recip = work.tile([64, T], F32, tag="recip")
        nc.vector.reciprocal(recip, scoreb)
        conf = work.tile([64, T], F32, tag="conf")
        nc.vector.tensor_tensor(out=conf, in0=recip, in1=maskt, op=OP.mult)
        nc.sync.dma_start(conf_d, conf)


def _get_compiled(T):
    key = ("nc", T)
    if key in _CACHE:
        return _CACHE[key]
    import concourse.bacc as bacc
    import concourse.tile as tile
    from concourse import mybir

    F32 = mybir.dt.float32
    U16 = mybir.dt.uint16
    I32 = mybir.dt.int32
    nc = bacc.Bacc("TRN2", target_bir_lowering=False, debug=False,
                   num_devices=_NCORES)

    ins_spec = [
        ("x2", [128, T, 32], F32),
        ("ne2", [64, T], F32),
        ("mask", [64, T], F32),
        ("trans_rep", [128, 32, 64], F32),
        ("S0", [128, 128], F32),
        ("S1", [128, 128], F32),
        ("tio_s", [64, 64], F32),
        ("tio_c", [64, 64], F32),
        ("iota", [64, 64], F32),
        ("trbB_own", [128, 32 - _CA, 32], F32),
        ("trbB_cross", [128, 32 - _CA, 32], F32),
    ]
    ins = tuple(
        nc.dram_tensor(name, shape, dt, kind="ExternalInput").ap()
        for name, shape, dt in ins_spec
    )
    outs = (
        nc.dram_tensor("tags", [64, T], I32, kind="ExternalOutput").ap(),
        nc.dram_tensor("conf", [64, T], F32, kind="ExternalOutput").ap(),
    )

    with tile.TileContext(nc) as tc:
        _build_tile_program(tc, outs, ins, T=T)
    nc.compile()
    _CACHE[key] = nc
    return nc


def _run(logits, transition_params, sequence_lengths, trace=False):
    from concourse.bass_utils import run_bass_kernel_spmd

    T = logits.shape[1]
    logits = np.asarray(logits, dtype=np.float32)
    trans = np.asarray(transition_params, dtype=np.float32)
    seq = np.asarray(sequence_lengths, dtype=np.int32)

    consts = _host_constants(trans)
    in_maps = []
    for c in range(_NCORES):
        sl = slice(c * _BL, (c + 1) * _BL)
        pc = _host_percore(logits[sl], seq[sl], T)
        m = {"x2": pc["x2"], "ne2": pc["ne2"], "mask": pc["mask"]}
        m.update(consts)
        in_maps.append(m)

    nc = _get_compiled(T)
    res = run_bass_kernel_spmd(nc, in_maps, list(range(_NCORES)),
                               trace=trace)
    tags = np.concatenate([np.asarray(res.results[c]["tags"])
                           for c in range(_NCORES)], axis=0)
    conf = np.concatenate([np.asarray(res.results[c]["conf"])
                           for c in range(_NCORES)], axis=0)
    return (tags.astype(np.int32), conf.astype(np.float32)), res


def kernel(logits, transition_params, sequence_lengths):
    (tags, conf), _ = _run(logits, transition_params, sequence_lengths)
    return tags, conf


# revision 5
# speedup vs baseline: 1.3707x; 1.0831x over previous
"""CRF Viterbi decode (nn_CRF, B=512 T=512 O=64) on 8 Trainium2 NeuronCores.

Pure data parallel: 64 sequences per core; the (64, 64) transition matrix and
derived constants are replicated.

Per-core layout: g = j_hi in {0,1}; partition p = g*64 + b; tag j = g*32+j_lo.

Forward (per step t): three engines build exact fp32 candidate blocks
concurrently; only DVE can max-reduce, so its reduce time is the floor:
  DVE:  hist = x2[:, t-1, :] + m2buf[:, t-1, :] (split-layout state); hX =
        cross-half partition swap of hist via two stream_shuffle ops with
        offset partition bases (HW-verified cross-quadrant moves); adds for
        columns 0..CA-1 from the PE-folded PSUM state; ALL segmented
        max-reduces -> m2buf[:, t, :].
  Pool: adds for columns CA..CA+CB-1 in per-partition [own, cross] i-order
        (max is order-invariant) reading hist/hX from SBUF (GPSIMD cannot
        touch PSUM and its TT ucode has no max).
  PE:   (a) psum_state(t) = S@x2 + S@m2 fold for DVE's columns; (b) builds
        ts for the last CP=8 columns in one PSUM bank: a K=2 group-
        indicator matmul lays down the trans block (the bank's one
        start=True), then identity-selector matmuls with broadcast rhs
        accumulate hist (own half) and hX (cross half).  Every element
        receives exactly one trans and one state contribution, so the PSUM
        accumulate is the same single fp32 add as the reference.

Backward, aligned in time t = T-1..0: cand_t = hist_t + trans[:, tag_{t+1}]
is built in PSUM from x2/m2 fold matmuls (pre-run, off the critical chain),
a Pool-computed hist window, and 4 one-hot h matmuls.  The argmax chain is
  max8 (top-8 values) -> max_index (first-index ties, = jnp.argmax)
  h = (iota == idx) * ne        (one tensor_scalar; ne=0 at t==L resets the
                                 chain so cand collapses to hist, reproducing
                                 init_tag/init_conf exactly)
  hBT = 32x32-blockwise DVE transpose, fixed up by straight/cross tables in
        the 4 K=32 h matmuls (PE quadrant layout as in the proven baseline).
max_index writes its 8 indices straight into tags8[:, t, :]; tags come from
tags8[:, :, 0] in the bulk epilogue.  Confidence = 1/sum exp(cand - max) via
ACT Exp with per-partition bias (bias = -max via a tiny tensor_scalar).

Hardware caveats kept from the previous session:
- matmul operands at partition base 64 crash the device (PE quadrant-3 bug)
  -- all contractions stay at base 0/32;
- start_tensor_calc=True lazily zeroes the whole per-partition 2KB PSUM
  region, so each accumulation group has exactly one start=True (a K=1
  zeroing matmul) and everything else accumulates.
"""
import numpy as np

_B, _T, _O = 512, 512, 64
_NCORES = 8
_BL = _B // _NCORES

_CA = 11           # DVE-adds tag columns (j_lo 0.._CA-1)
_CB = 13          # Pool-adds tag columns (j_lo _CA.._CA+_CB-1)
_CP = 8           # PE-built tag columns (j_lo _CA+_CB..31), one psum bank
_USE_MAXIDX = True  # max8/max_index argmax vs baseline 5-op argmax

_CACHE = {}


def _host_constants(trans):
    trans = np.ascontiguousarray(trans.astype(np.float32))
    transT = np.ascontiguousarray(trans.T)                  # [j, i]
    tr = transT.reshape(2, 32, 64)
    trans_rep = np.ascontiguousarray(
        np.broadcast_to(tr[:, None, :, :], (2, 64, 32, 64)).reshape(128, 32, 64)
    )
    S = np.zeros((2, 128, 128), np.float32)
    for h in range(2):
        for b in range(64):
            S[h, h * 64 + b, b] = 1.0
            S[h, h * 64 + b, 64 + b] = 1.0
    tio_s = np.ascontiguousarray(transT)                    # [64, 64]
    tio_c = np.ascontiguousarray(
        np.concatenate([tio_s[32:64], tio_s[0:32]], axis=0))
    iota = np.ascontiguousarray(
        np.broadcast_to(np.arange(64, dtype=np.float32), (64, 64)))
    # Pool B-column tables with per-partition [own-half, cross-half] i order:
    # trbB_own[p=(g,b), j_lo, i_lo] = trans[g*32+i_lo, g*32+(CA+j_lo)]
    # trbB_cross[p, j_lo, i_lo]     = trans[(1-g)*32+i_lo, g*32+(CA+j_lo)]
    CA, CB, CP = _CA, _CB, _CP
    g = (np.arange(128) // 64)[:, None, None]           # [128,1,1]
    jl = (CA + np.arange(CB))[None, :, None]            # [1,CB,1]
    il = np.arange(32)[None, None, :]                   # [1,1,32]
    trbB_own = np.ascontiguousarray(
        trans[g * 32 + il, g * 32 + jl].astype(np.float32))
    trbB_cross = np.ascontiguousarray(
        trans[(1 - g) * 32 + il, g * 32 + jl].astype(np.float32))
    I128 = np.eye(128, dtype=np.float32)
    # PE column block (j_lo = CA+CB..31): K=2 indicator rows select the
    # per-partition-group trans table; i-axis in [own, cross] order.
    gi = np.arange(2)[:, None, None]
    jp = (CA + CB + np.arange(CP))[None, :, None]
    trans8 = np.zeros((2, CP, 64), np.float32)
    trans8[:, :, 0:32] = trans[gi * 32 + il, gi * 32 + jp]
    trans8[:, :, 32:64] = trans[(1 - gi) * 32 + il, gi * 32 + jp]
    trans8 = np.ascontiguousarray(trans8.reshape(2, CP * 64))
    ones2 = np.zeros((2, 128), np.float32)
    ones2[0, 0:64] = 1.0
    ones2[1, 64:128] = 1.0
    return {
        "trans_rep": trans_rep,
        "S0": np.ascontiguousarray(S[0]),
        "S1": np.ascontiguousarray(S[1]),
        "tio_s": tio_s,
        "tio_c": tio_c,
        "iota": iota,
        "trbB_own": trbB_own,
        "trbB_cross": trbB_cross,
        "I128": I128,
        "trans8": trans8,
        "ones2": ones2,
    }


def _host_percore(logits_c, seq_c, T):
    x2 = np.ascontiguousarray(
        logits_c.astype(np.float32)
        .reshape(_BL, T, 2, 32).transpose(2, 0, 1, 3).reshape(128, T, 32)
    )
    # ne2[b, t] = 0 iff t == L_b: at backward step t == L the one-hot is
    # zeroed so cand_{L-1} collapses to hist_{L-1} (the reference's frozen
    # last_score).
    ne2 = np.ones((_BL, T), np.float32)
    sel = seq_c <= T - 1
    ne2[np.arange(_BL)[sel], seq_c[sel]] = 0.0
    mask = (np.arange(T)[None, :] < seq_c[:, None]).astype(np.float32)
    return {"x2": x2, "ne2": ne2, "mask": mask,
            "ne2u": ne2.astype(np.uint16)}


def _build_tile_program(tc, outs, ins, T):
    from contextlib import ExitStack
    import concourse.bass as bass
    from concourse import mybir
    from concourse.tile import add_dep_helper

    F32 = mybir.dt.float32
    U16 = mybir.dt.uint16
    AX = mybir.AxisListType
    OP = mybir.AluOpType
    ACT = mybir.ActivationFunctionType

    nc = tc.nc
    tags_d, conf_d = outs
    (x2_d, ne2_d, mask_d, transrep_d, s0_d, s1_d, tios_d, tioc_d,
     iota_d, trbo_d, trbc_d, i128_d, trans8_d, ones2_d) = ins

    CA, CB, CP = _CA, _CB, _CP

    def bcast_mid(ap2d, n):
        assert len(ap2d.ap) == 2, ap2d.ap
        return bass.AP(tensor=ap2d.tensor, offset=ap2d.offset,
                       ap=[ap2d.ap[0], [0, n], ap2d.ap[1]])

    def chain_mms(insts):
        for a, b in zip(insts[1:], insts[:-1]):
            add_dep_helper(a.ins, b.ins, sync=False,
                           reason="psum accumulation order")
        return insts[-1]

    with ExitStack() as ctx:
        consts = ctx.enter_context(tc.tile_pool(name="consts", bufs=1))
        big = ctx.enter_context(tc.tile_pool(name="big", bufs=1))
        work = ctx.enter_context(tc.tile_pool(name="work", bufs=2))
        tsA_p = ctx.enter_context(tc.tile_pool(name="tsA", bufs=2))
        tsB_p = ctx.enter_context(tc.tile_pool(name="tsB", bufs=2))
        histw = ctx.enter_context(tc.tile_pool(name="histw", bufs=4))
        pstA = ctx.enter_context(
            tc.tile_pool(name="pstA", bufs=2, space="PSUM"))
        ts8p = ctx.enter_context(
            tc.tile_pool(name="ts8p", bufs=2, space="PSUM"))
        pbw = ctx.enter_context(
            tc.tile_pool(name="pbw", bufs=3, space="PSUM"))
        hxp = ctx.enter_context(tc.tile_pool(name="hxp", bufs=2))

        trans_rep = consts.tile([128, 32, 64], F32)
        nc.sync.dma_start(trans_rep, transrep_d)
        S0 = consts.tile([128, 128], F32)
        nc.sync.dma_start(S0, s0_d)
        S1 = consts.tile([128, 128], F32)
        nc.sync.dma_start(S1, s1_d)
        tio_s = consts.tile([64, 64], F32)
        nc.sync.dma_start(tio_s, tios_d)
        tio_c = consts.tile([64, 64], F32)
        nc.sync.dma_start(tio_c, tioc_d)
        iota = consts.tile([64, 64], F32)
        nc.sync.dma_start(iota, iota_d)
        trbB_own = consts.tile([128, CB, 32], F32)
        nc.sync.dma_start(trbB_own, trbo_d)
        trbB_cross = consts.tile([128, CB, 32], F32)
        nc.sync.dma_start(trbB_cross, trbc_d)
        I128 = consts.tile([128, 128], F32)
        nc.sync.dma_start(I128, i128_d)
        trans8 = consts.tile([2, CP * 64], F32)
        nc.sync.dma_start(trans8, trans8_d)
        ones2 = consts.tile([2, 128], F32)
        nc.sync.dma_start(ones2, ones2_d)
        ne2 = consts.tile([64, T], F32)
        nc.sync.dma_start(ne2, ne2_d)
        maskt = consts.tile([64, T], F32)
        nc.sync.dma_start(maskt, mask_d)
        zl = consts.tile([1, 64], F32)
        nc.vector.memset(zl, 0.0)
        zl128 = consts.tile([1, 128], F32)
        nc.vector.memset(zl128, 0.0)
        zr128 = consts.tile([1, 128], F32)
        nc.vector.memset(zr128, 0.0)
        zr = consts.tile([1, 64], F32)
        nc.vector.memset(zr, 0.0)

        x2 = big.tile([128, T, 32], F32)
        NCH = 8
        CT = T // NCH
        for c in range(NCH):
            nc.sync.dma_start(x2[:, c * CT:(c + 1) * CT, :],
                              x2_d[:, c * CT:(c + 1) * CT, :])
        m2buf = big.tile([128, T, 32], F32)
        tags8 = big.tile([64, T, 8], U16)
        scoreb = big.tile([64, T], F32)

        # ---------------- forward ----------------
        # Division of labor (hardware constraints: GPSIMD cannot access
        # PSUM, and its software TensorTensor only implements add/mult —
        # no max): DVE reads the PSUM state directly and handles the adds
        # for columns 0..CA-1 plus ALL segmented max-reduces; Pool adds
        # columns CA..31 from an SBUF state copy made by the otherwise-idle
        # ACT engine.
        def fwd_fold(ps, t):
            """psum_state(t) = S@x2[:, t, :] (+ S@m2buf[:, t, :] if t>0)."""
            i0 = nc.tensor.matmul(ps[:, :], zl128, zr, start=True,
                                  stop=False, skip_group_check=True)
            mms = [i0]
            mms.append(nc.tensor.matmul(ps[:, 0:32], S0, x2[:, t, :],
                                        start=False, stop=False,
                                        skip_group_check=True))
            mms.append(nc.tensor.matmul(ps[:, 32:64], S1, x2[:, t, :],
                                        start=False, stop=(t == 0),
                                        skip_group_check=True))
            if t > 0:
                mms.append(nc.tensor.matmul(
                    ps[:, 0:CA], S0, m2buf[:, t, 0:CA],
                    start=False, stop=False, skip_group_check=True))
                mms.append(nc.tensor.matmul(
                    ps[:, 32:32 + CA], S1, m2buf[:, t, 0:CA],
                    start=False, stop=False, skip_group_check=True))
                mms.append(nc.tensor.matmul(
                    ps[:, CA:32], S0, m2buf[:, t, CA:32],
                    start=False, stop=False, skip_group_check=True))
                mms.append(nc.tensor.matmul(
                    ps[:, 32 + CA:64], S1, m2buf[:, t, CA:32],
                    start=False, stop=True, skip_group_check=True))
            chain_mms(mms)
            return ps

        psA_prev = pstA.tile([128, 64], F32, tag="psA")
        fwd_fold(psA_prev, 0)

        ident = list(range(32))
        for t in range(1, T):
            # DVE: previous state in split layout (own half per partition)
            # plus its cross-half partition swap, feeding Pool from SBUF.
            if t == 1:
                hist = x2[:, 0, :]
            else:
                histt = hxp.tile([128, 32], F32, tag="hist")
                nc.vector.tensor_tensor(out=histt, in0=x2[:, t - 1, :],
                                        in1=m2buf[:, t - 1, :], op=OP.add)
                hist = histt[:]
            hX = hxp.tile([128, 32], F32, tag="hX")
            nc.vector.stream_shuffle(hX[0:64, :], hist[64:128, :], ident)
            nc.vector.stream_shuffle(hX[64:128, :], hist[0:64, :], ident)
            # Pool: adds for columns CA..CA+CB-1 in [own, cross] i-order
            tsB = tsB_p.tile([128, CB, 64], F32, tag="tsB")
            nc.gpsimd.tensor_tensor(
                out=tsB[:, :, 0:32], in0=trbB_own,
                in1=bass.AP(tensor=hist.tensor, offset=hist.offset,
                            ap=[hist.ap[0], [0, CB], hist.ap[1]]),
                op=OP.add)
            nc.gpsimd.tensor_tensor(
                out=tsB[:, :, 32:64], in0=trbB_cross,
                in1=bcast_mid(hX[:], CB), op=OP.add)
            # PE: builds ts for columns CA+CB..31 in PSUM: the constant
            # trans block (start=True, K=2 group indicator), then +hist and
            # +hX via identity selectors with broadcast rhs.  Each element
            # receives exactly one trans value and one state value, so the
            # PSUM accumulation is the same single fp32 add as the reference.
            ts8 = ts8p.tile([128, CP, 64], F32, tag="ts8")
            p0 = nc.tensor.matmul(
                bass.AP(tensor=ts8.tensor, offset=ts8.offset,
                        ap=[ts8.ap[0], [1, CP * 64]]),
                ones2, trans8, start=True, stop=False, skip_group_check=True)
            p1 = nc.tensor.matmul(
                ts8[:, :, 0:32], I128,
                bass.AP(tensor=hist.tensor, offset=hist.offset,
                        ap=[hist.ap[0], [0, CP], hist.ap[1]]),
                start=False, stop=False, skip_group_check=True)
            p2 = nc.tensor.matmul(
                ts8[:, :, 32:64], I128, bcast_mid(hX[:], CP),
                start=False, stop=True, skip_group_check=True)
            chain_mms([p0, p1, p2])
            # DVE: adds for columns 0..CA-1, then the three segmented reduces
            tsA = tsA_p.tile([128, CA, 64], F32, tag="tsA")
            nc.vector.tensor_tensor(
                out=tsA, in0=trans_rep[:, 0:CA, :],
                in1=bcast_mid(psA_prev[:], CA), op=OP.add)
            nc.vector.tensor_reduce(m2buf[:, t, 0:CA], tsA,
                                    axis=AX.X, op=OP.max)
            nc.vector.tensor_reduce(m2buf[:, t, CA + CB:32], ts8[:],
                                    axis=AX.X, op=OP.max)
            nc.vector.tensor_reduce(m2buf[:, t, CA:CA + CB], tsB,
                                    axis=AX.X, op=OP.max)

            psA = pstA.tile([128, 64], F32, tag="psA")
            fwd_fold(psA, t)
            psA_prev = psA

        # ---------------- backward ----------------
        def bwd_prep(ps, t, hb):
            """cand base: ps = hist_t replicated over i-columns."""
            i0 = nc.tensor.matmul(ps[:, :], zl, zr, start=True, stop=False,
                                  skip_group_check=True)
            i1 = nc.tensor.matmul(ps[:, 0:32], S0[:, 0:64], hb,
                                  start=False, stop=False,
                                  skip_group_check=True)
            i2 = nc.tensor.matmul(ps[:, 32:64], S1[:, 0:64], hb,
                                  start=False, stop=(t == T - 1),
                                  skip_group_check=True)
            return chain_mms([i0, i1, i2])

        def h_mms(ps, hBT, after):
            i1 = nc.tensor.matmul(ps[0:32, :], hBT[0:32, 0:32],
                                  tio_s[0:32, :], start=False, stop=False,
                                  skip_group_check=True)
            i2 = nc.tensor.matmul(ps[0:32, :], hBT[0:32, 32:64],
                                  tio_c[0:32, :], start=False, stop=False,
                                  skip_group_check=True)
            i3 = nc.tensor.matmul(ps[32:64, :], hBT[32:64, 0:32],
                                  tio_c[32:64, :], start=False, stop=False,
                                  skip_group_check=True)
            i4 = nc.tensor.matmul(ps[32:64, :], hBT[32:64, 32:64],
                                  tio_s[32:64, :], start=False, stop=True,
                                  skip_group_check=True)
            return chain_mms([after, i1, i2, i3, i4])

        def hist_tile(t):
            if t == 0:
                return x2[:, 0, :]  # init state: m2 only exists for t >= 1
            hb = histw.tile([128, 32], F32, tag="histw")
            nc.gpsimd.tensor_tensor(out=hb, in0=x2[:, t, :],
                                    in1=m2buf[:, t, :], op=OP.add)
            return hb

        def bwd_dve(ps, t):
            m8 = work.tile([64, 8], F32, tag="m8")
            nc.vector.max(m8, ps[:])
            nc.vector.max_index(tags8[:, t, :], m8, ps[:])
            idxf = work.tile([64, 1], F32, tag="idxf")
            nc.vector.tensor_copy(idxf, tags8[:, t, 0:1])
            h = work.tile([64, 64], F32, tag="h")
            nc.vector.tensor_scalar(out=h, in0=iota,
                                    scalar1=idxf,
                                    scalar2=ne2[:, t:t + 1],
                                    op0=OP.is_equal, op1=OP.mult)
            hBT = work.tile([64, 64], F32, tag="hBT")
            nc.vector.transpose(hBT, h)
            negmax = work.tile([64, 1], F32, tag="negmax")
            nc.vector.tensor_scalar(out=negmax, in0=m8[:, 0:1],
                                    scalar1=-1.0, scalar2=None, op0=OP.mult)
            e = work.tile([64, 64], F32, tag="e")
            nc.scalar.activation(out=e, in_=ps[:], func=ACT.Exp,
                                 bias=negmax, scale=1.0,
                                 accum_out=scoreb[:, t:t + 1])
            return hBT

        ps = pbw.tile([64, 64], F32, tag="bwps")
        bwd_prep(ps, T - 1, hist_tile(T - 1))
        hBT = bwd_dve(ps, T - 1)

        for t in range(T - 2, -1, -1):
            ps = pbw.tile([64, 64], F32, tag="bwps")
            last = bwd_prep(ps, t, hist_tile(t))
            h_mms(ps, hBT, after=last)
            hBT = bwd_dve(ps, t)

        # ---------------- epilogue ----------------
        tagsf = work.tile([64, T], F32, tag="tagsf")
        t8v = bass.AP(tensor=tags8.tensor, offset=tags8.offset,
                      ap=[tags8.ap[0], [8, T]])
        nc.vector.tensor_copy(tagsf, t8v)
        tagsm = work.tile([64, T], F32, tag="tagsm")
        nc.vector.tensor_tensor(out=tagsm, in0=tagsf, in1=maskt, op=OP.mult)
        tagsi = work.tile([64, T], mybir.dt.int32, tag="tagsi")
        nc.vector.tensor_copy(tagsi, tagsm)
        nc.sync.dma_start(tags_d, tagsi)
        recip = work.tile([64, T], F32, tag="recip")
        nc.vector.reciprocal(recip, scoreb)
        conf = work.tile([64, T], F32, tag="conf")
        nc.vector.tensor_tensor(out=conf, in0=recip, in1=maskt, op=OP.mult)
        nc.sync.dma_start(conf_d, conf)


def _get_compiled(T):
    key = ("nc", T)
    if key in _CACHE:
        return _CACHE[key]
    import concourse.bacc as bacc
    import concourse.tile as tile
    from concourse import mybir

    F32 = mybir.dt.float32
    U16 = mybir.dt.uint16
    I32 = mybir.dt.int32
    nc = bacc.Bacc("TRN2", target_bir_lowering=False, debug=False,
                   num_devices=_NCORES)

    ins_spec = [
        ("x2", [128, T, 32], F32),
        ("ne2", [64, T], F32),
        ("mask", [64, T], F32),
        ("trans_rep", [128, 32, 64], F32),
        ("S0", [128, 128], F32),
        ("S1", [128, 128], F32),
        ("tio_s", [64, 64], F32),
        ("tio_c", [64, 64], F32),
        ("iota", [64, 64], F32),
        ("trbB_own", [128, _CB, 32], F32),
        ("trbB_cross", [128, _CB, 32], F32),
        ("I128", [128, 128], F32),
        ("trans8", [2, _CP * 64], F32),
        ("ones2", [2, 128], F32),
    ]
    ins = tuple(
        nc.dram_tensor(name, shape, dt, kind="ExternalInput").ap()
        for name, shape, dt in ins_spec
    )
    outs = (
        nc.dram_tensor("tags", [64, T], I32, kind="ExternalOutput").ap(),
        nc.dram_tensor("conf", [64, T], F32, kind="ExternalOutput").ap(),
    )

    with tile.TileContext(nc) as tc:
        _build_tile_program(tc, outs, ins, T=T)
    nc.compile()
    _CACHE[key] = nc
    return nc


def _run(logits, transition_params, sequence_lengths, trace=False):
    from concourse.bass_utils import run_bass_kernel_spmd

    T = logits.shape[1]
    logits = np.asarray(logits, dtype=np.float32)
    trans = np.asarray(transition_params, dtype=np.float32)
    seq = np.asarray(sequence_lengths, dtype=np.int32)

    consts = _host_constants(trans)
    in_maps = []
    for c in range(_NCORES):
        sl = slice(c * _BL, (c + 1) * _BL)
        pc = _host_percore(logits[sl], seq[sl], T)
        m = {"x2": pc["x2"], "ne2": pc["ne2"], "mask": pc["mask"]}
        m.update(consts)
        in_maps.append(m)

    nc = _get_compiled(T)
    res = run_bass_kernel_spmd(nc, in_maps, list(range(_NCORES)),
                               trace=trace)
    tags = np.concatenate([np.asarray(res.results[c]["tags"])
                           for c in range(_NCORES)], axis=0)
    conf = np.concatenate([np.asarray(res.results[c]["conf"])
                           for c in range(_NCORES)], axis=0)
    return (tags.astype(np.int32), conf.astype(np.float32)), res


def kernel(logits, transition_params, sequence_lengths):
    (tags, conf), _ = _run(logits, transition_params, sequence_lengths)
    return tags, conf


# revision 8
# speedup vs baseline: 1.3928x; 1.0161x over previous
"""CRF Viterbi decode (nn_CRF, B=512 T=512 O=64) on 8 Trainium2 NeuronCores.

Pure data parallel: 64 sequences per core; the (64, 64) transition matrix and
derived constants are replicated.

Per-core layout: g = j_hi in {0,1}; partition p = g*64 + b; tag j = g*32+j_lo.

Forward (per step t): three engines build exact fp32 candidate blocks
concurrently; only DVE can max-reduce, so its reduce time is the floor:
  DVE:  hist = x2[:, t-1, :] + m2buf[:, t-1, :] (split-layout state); hX =
        cross-half partition swap of hist via two stream_shuffle ops with
        offset partition bases (HW-verified cross-quadrant moves); adds for
        columns 0..CA-1 in [own, cross] halves straight from hist/hX in
        SBUF (no PSUM state fold at all); ALL segmented max-reduces
        -> m2buf[:, t, :].
  Pool: adds for columns CA..CA+CB-1 in per-partition [own, cross] i-order
        (max is order-invariant) reading hist/hX from SBUF (GPSIMD cannot
        touch PSUM and its TT ucode has no max).
  PE:   builds ts for the last CP=8 columns in one PSUM bank: a K=2
        group-indicator matmul lays down the trans block (the bank's one
        start=True), then identity-selector matmuls with broadcast rhs
        accumulate hist (own half) and hX (cross half).  Every element
        receives exactly one trans and one state contribution, so the PSUM
        accumulate is the same single fp32 add as the reference.

Backward, aligned in time t = T-1..0: cand_t = hist_t + trans[:, tag_{t+1}]
is built in PSUM from x2/m2 fold matmuls (pre-run, off the critical chain),
a Pool-computed hist window, and 4 one-hot h matmuls.  The argmax chain is
  max8 (top-8 values) -> max_index (first-index ties, = jnp.argmax)
  h = (iota == idx) * ne        (one tensor_scalar; ne=0 at t==L resets the
                                 chain so cand collapses to hist, reproducing
                                 init_tag/init_conf exactly)
  hBT = 32x32-blockwise DVE transpose, fixed up by straight/cross tables in
        the 4 K=32 h matmuls (PE quadrant layout as in the proven baseline).
max_index writes its 8 indices straight into tags8[:, t, :]; tags come from
tags8[:, :, 0] in the bulk epilogue.  Confidence = 1/sum exp(cand - max) via
ACT Exp with per-partition bias (bias = -max via a tiny tensor_scalar).

Hardware caveats kept from the previous session:
- matmul operands at partition base 64 crash the device (PE quadrant-3 bug)
  -- all contractions stay at base 0/32;
- start_tensor_calc=True lazily zeroes the whole per-partition 2KB PSUM
  region, so each accumulation group has exactly one start=True (a K=1
  zeroing matmul) and everything else accumulates.
"""
import numpy as np

_B, _T, _O = 512, 512, 64
_NCORES = 8
_BL = _B // _NCORES

_CA = 11           # DVE-adds tag columns (j_lo 0.._CA-1)
_CB = 13          # Pool-adds tag columns (j_lo _CA.._CA+_CB-1)
_CP = 8           # PE-built tag columns (j_lo _CA+_CB..31), one psum bank
_USE_MAXIDX = True  # max8/max_index argmax vs baseline 5-op argmax

_CACHE = {}


def _host_constants(trans):
    trans = np.ascontiguousarray(trans.astype(np.float32))
    transT = np.ascontiguousarray(trans.T)                  # [j, i]
    tr = transT.reshape(2, 32, 64)
    trans_rep = np.ascontiguousarray(
        np.broadcast_to(tr[:, None, :, :], (2, 64, 32, 64)).reshape(128, 32, 64)
    )
    S = np.zeros((2, 128, 128), np.float32)
    for h in range(2):
        for b in range(64):
            S[h, h * 64 + b, b] = 1.0
            S[h, h * 64 + b, 64 + b] = 1.0
    tio_s = np.ascontiguousarray(transT)                    # [64, 64]
    tio_c = np.ascontiguousarray(
        np.concatenate([tio_s[32:64], tio_s[0:32]], axis=0))
    iota = np.ascontiguousarray(
        np.broadcast_to(np.arange(64, dtype=np.float32), (64, 64)))
    # Pool B-column tables with per-partition [own-half, cross-half] i order:
    # trbB_own[p=(g,b), j_lo, i_lo] = trans[g*32+i_lo, g*32+(CA+j_lo)]
    # trbB_cross[p, j_lo, i_lo]     = trans[(1-g)*32+i_lo, g*32+(CA+j_lo)]
    CA, CB, CP = _CA, _CB, _CP
    g = (np.arange(128) // 64)[:, None, None]           # [128,1,1]
    jl = np.arange(CA + CB)[None, :, None]              # [1,CA+CB,1]
    il = np.arange(32)[None, None, :]                   # [1,1,32]
    trbB_own = np.ascontiguousarray(
        trans[g * 32 + il, g * 32 + jl].astype(np.float32))
    trbB_cross = np.ascontiguousarray(
        trans[(1 - g) * 32 + il, g * 32 + jl].astype(np.float32))
    I128 = np.eye(128, dtype=np.float32)
    # PE column block (j_lo = CA+CB..31): K=2 indicator rows select the
    # per-partition-group trans table; i-axis in [own, cross] order.
    gi = np.arange(2)[:, None, None]
    jp = (CA + CB + np.arange(CP))[None, :, None]
    trans8 = np.zeros((2, CP, 64), np.float32)
    trans8[:, :, 0:32] = trans[gi * 32 + il, gi * 32 + jp]
    trans8[:, :, 32:64] = trans[(1 - gi) * 32 + il, gi * 32 + jp]
    trans8 = np.ascontiguousarray(trans8.reshape(2, CP * 64))
    ones2 = np.zeros((2, 128), np.float32)
    ones2[0, 0:64] = 1.0
    ones2[1, 64:128] = 1.0
    return {
        "trans_rep": trans_rep,
        "S0": np.ascontiguousarray(S[0]),
        "S1": np.ascontiguousarray(S[1]),
        "tio_s": tio_s,
        "tio_c": tio_c,
        "iota": iota,
        "trbB_own": trbB_own,
        "trbB_cross": trbB_cross,
        "I128": I128,
        "trans8": trans8,
        "ones2": ones2,
    }


def _host_percore(logits_c, seq_c, T):
    x2 = np.ascontiguousarray(
        logits_c.astype(np.float32)
        .reshape(_BL, T, 2, 32).transpose(2, 0, 1, 3).reshape(128, T, 32)
    )
    # ne2[b, t] = 0 iff t == L_b: at backward step t == L the one-hot is
    # zeroed so cand_{L-1} collapses to hist_{L-1} (the reference's frozen
    # last_score).
    ne2 = np.ones((_BL, T), np.float32)
    sel = seq_c <= T - 1
    ne2[np.arange(_BL)[sel], seq_c[sel]] = 0.0
    mask = (np.arange(T)[None, :] < seq_c[:, None]).astype(np.float32)
    return {"x2": x2, "ne2": ne2, "mask": mask,
            "ne2u": ne2.astype(np.uint16)}


def _build_tile_program(tc, outs, ins, T):
    from contextlib import ExitStack
    import concourse.bass as bass
    from concourse import mybir
    from concourse.tile import add_dep_helper

    F32 = mybir.dt.float32
    U16 = mybir.dt.uint16
    AX = mybir.AxisListType
    OP = mybir.AluOpType
    ACT = mybir.ActivationFunctionType

    nc = tc.nc
    tags_d, conf_d = outs
    (x2_d, ne2_d, mask_d, transrep_d, s0_d, s1_d, tios_d, tioc_d,
     iota_d, trbo_d, trbc_d, i128_d, trans8_d, ones2_d) = ins

    CA, CB, CP = _CA, _CB, _CP

    def bcast_mid(ap2d, n):
        assert len(ap2d.ap) == 2, ap2d.ap
        return bass.AP(tensor=ap2d.tensor, offset=ap2d.offset,
                       ap=[ap2d.ap[0], [0, n], ap2d.ap[1]])

    def chain_mms(insts):
        for a, b in zip(insts[1:], insts[:-1]):
            add_dep_helper(a.ins, b.ins, sync=False,
                           reason="psum accumulation order")
        return insts[-1]

    with ExitStack() as ctx:
        consts = ctx.enter_context(tc.tile_pool(name="consts", bufs=1))
        big = ctx.enter_context(tc.tile_pool(name="big", bufs=1))
        work = ctx.enter_context(tc.tile_pool(name="work", bufs=2))
        tsA_p = ctx.enter_context(tc.tile_pool(name="tsA", bufs=2))
        tsB_p = ctx.enter_context(tc.tile_pool(name="tsB", bufs=2))
        histw = ctx.enter_context(tc.tile_pool(name="histw", bufs=4))
        ts8p = ctx.enter_context(
            tc.tile_pool(name="ts8p", bufs=2, space="PSUM"))
        pbw = ctx.enter_context(
            tc.tile_pool(name="pbw", bufs=3, space="PSUM"))
        hxp = ctx.enter_context(tc.tile_pool(name="hxp", bufs=2))

        trans_rep = consts.tile([128, 32, 64], F32)
        nc.sync.dma_start(trans_rep, transrep_d)
        S0 = consts.tile([128, 128], F32)
        nc.sync.dma_start(S0, s0_d)
        S1 = consts.tile([128, 128], F32)
        nc.sync.dma_start(S1, s1_d)
        tio_s = consts.tile([64, 64], F32)
        nc.sync.dma_start(tio_s, tios_d)
        tio_c = consts.tile([64, 64], F32)
        nc.sync.dma_start(tio_c, tioc_d)
        iota = consts.tile([64, 64], F32)
        nc.sync.dma_start(iota, iota_d)
        trbB_own = consts.tile([128, CA + CB, 32], F32)
        nc.sync.dma_start(trbB_own, trbo_d)
        trbB_cross = consts.tile([128, CA + CB, 32], F32)
        nc.sync.dma_start(trbB_cross, trbc_d)
        I128 = consts.tile([128, 128], F32)
        nc.sync.dma_start(I128, i128_d)
        trans8 = consts.tile([2, CP * 64], F32)
        nc.sync.dma_start(trans8, trans8_d)
        ones2 = consts.tile([2, 128], F32)
        nc.sync.dma_start(ones2, ones2_d)
        ne2 = consts.tile([64, T], F32)
        nc.sync.dma_start(ne2, ne2_d)
        maskt = consts.tile([64, T], F32)
        nc.sync.dma_start(maskt, mask_d)
        zl = consts.tile([1, 64], F32)
        nc.vector.memset(zl, 0.0)
        zl128 = consts.tile([1, 128], F32)
        nc.vector.memset(zl128, 0.0)
        zr128 = consts.tile([1, 128], F32)
        nc.vector.memset(zr128, 0.0)
        zr = consts.tile([1, 64], F32)
        nc.vector.memset(zr, 0.0)

        x2 = big.tile([128, T, 32], F32)
        NCH = 8
        CT = T // NCH
        for c in range(NCH):
            nc.sync.dma_start(x2[:, c * CT:(c + 1) * CT, :],
                              x2_d[:, c * CT:(c + 1) * CT, :])
        m2buf = big.tile([128, T, 32], F32)
        tags8 = big.tile([64, T, 8], U16)
        scoreb = big.tile([64, T], F32)

        # ---------------- forward ----------------
        # Division of labor (hardware constraints: GPSIMD cannot access
        # PSUM, and its software TensorTensor only implements add/mult —
        # no max): DVE reads the PSUM state directly and handles the adds
        # for columns 0..CA-1 plus ALL segmented max-reduces; Pool adds
        # columns CA..31 from an SBUF state copy made by the otherwise-idle
        # ACT engine.
        ident = list(range(32))
        for t in range(1, T):
            # DVE: previous state in split layout (own half per partition)
            # plus its cross-half partition swap, feeding Pool from SBUF.
            if t == 1:
                hist = x2[:, 0, :]
            else:
                histt = hxp.tile([128, 32], F32, tag="hist")
                nc.vector.tensor_tensor(out=histt, in0=x2[:, t - 1, :],
                                        in1=m2buf[:, t - 1, :], op=OP.add)
                hist = histt[:]
            hX = hxp.tile([128, 32], F32, tag="hX")
            nc.vector.stream_shuffle(hX[0:64, :], hist[64:128, :], ident)
            nc.vector.stream_shuffle(hX[64:128, :], hist[0:64, :], ident)
            def hist_b(n):
                return bass.AP(tensor=hist.tensor, offset=hist.offset,
                               ap=[hist.ap[0], [0, n], hist.ap[1]])

            # Pool: adds for columns CA..CA+CB-1 in [own, cross] i-order
            tsB = tsB_p.tile([128, CB, 64], F32, tag="tsB")
            nc.gpsimd.tensor_tensor(
                out=tsB[:, :, 0:32], in0=trbB_own[:, CA:CA + CB, :],
                in1=hist_b(CB), op=OP.add)
            nc.gpsimd.tensor_tensor(
                out=tsB[:, :, 32:64], in0=trbB_cross[:, CA:CA + CB, :],
                in1=bcast_mid(hX[:], CB), op=OP.add)
            # PE: builds ts for columns CA+CB..31 in PSUM: the constant
            # trans block (start=True, K=2 group indicator), then +hist and
            # +hX via identity selectors with broadcast rhs.  Each element
            # receives exactly one trans value and one state value, so the
            # PSUM accumulation is the same single fp32 add as the reference.
            ts8 = ts8p.tile([128, CP, 64], F32, tag="ts8")
            p0 = nc.tensor.matmul(
                bass.AP(tensor=ts8.tensor, offset=ts8.offset,
                        ap=[ts8.ap[0], [1, CP * 64]]),
                ones2, trans8, start=True, stop=False, skip_group_check=True)
            p1 = nc.tensor.matmul(
                ts8[:, :, 0:32], I128, hist_b(CP),
                start=False, stop=False, skip_group_check=True)
            p2 = nc.tensor.matmul(
                ts8[:, :, 32:64], I128, bcast_mid(hX[:], CP),
                start=False, stop=True, skip_group_check=True)
            chain_mms([p0, p1, p2])
            # DVE: adds for columns 0..CA-1 (own/cross halves from SBUF),
            # then the three segmented reduces
            tsA = tsA_p.tile([128, CA, 64], F32, tag="tsA")
            nc.vector.tensor_tensor(
                out=tsA[:, :, 0:32], in0=trbB_own[:, 0:CA, :],
                in1=hist_b(CA), op=OP.add)
            nc.vector.tensor_tensor(
                out=tsA[:, :, 32:64], in0=trbB_cross[:, 0:CA, :],
                in1=bcast_mid(hX[:], CA), op=OP.add)
            nc.vector.tensor_reduce(m2buf[:, t, 0:CA], tsA,
                                    axis=AX.X, op=OP.max)
            nc.vector.tensor_reduce(m2buf[:, t, CA + CB:32], ts8[:],
                                    axis=AX.X, op=OP.max)
            nc.vector.tensor_reduce(m2buf[:, t, CA:CA + CB], tsB,
                                    axis=AX.X, op=OP.max)

        # ---------------- backward ----------------
        def bwd_prep(ps, t, hb):
            """cand base: ps = hist_t replicated over i-columns."""
            i0 = nc.tensor.matmul(ps[:, :], zl, zr, start=True, stop=False,
                                  skip_group_check=True)
            i1 = nc.tensor.matmul(ps[:, 0:32], S0[:, 0:64], hb,
                                  start=False, stop=False,
                                  skip_group_check=True)
            i2 = nc.tensor.matmul(ps[:, 32:64], S1[:, 0:64], hb,
                                  start=False, stop=(t == T - 1),
                                  skip_group_check=True)
            return chain_mms([i0, i1, i2])

        def h_mms(ps, hBT, after):
            i1 = nc.tensor.matmul(ps[0:32, :], hBT[0:32, 0:32],
                                  tio_s[0:32, :], start=False, stop=False,
                                  skip_group_check=True)
            i2 = nc.tensor.matmul(ps[0:32, :], hBT[0:32, 32:64],
                                  tio_c[0:32, :], start=False, stop=False,
                                  skip_group_check=True)
            i3 = nc.tensor.matmul(ps[32:64, :], hBT[32:64, 0:32],
                                  tio_c[32:64, :], start=False, stop=False,
                                  skip_group_check=True)
            i4 = nc.tensor.matmul(ps[32:64, :], hBT[32:64, 32:64],
                                  tio_s[32:64, :], start=False, stop=True,
                                  skip_group_check=True)
            return chain_mms([after, i1, i2, i3, i4])

        def hist_tile(t):
            if t == 0:
                return x2[:, 0, :]  # init state: m2 only exists for t >= 1
            hb = histw.tile([128, 32], F32, tag="histw")
            nc.gpsimd.tensor_tensor(out=hb, in0=x2[:, t, :],
                                    in1=m2buf[:, t, :], op=OP.add)
            return hb

        def bwd_dve(ps, t):
            m8 = work.tile([64, 8], F32, tag="m8")
            nc.vector.max(m8, ps[:])
            nc.vector.max_index(tags8[:, t, :], m8, ps[:])
            idxf = work.tile([64, 1], F32, tag="idxf")
            nc.vector.tensor_copy(idxf, tags8[:, t, 0:1])
            h = work.tile([64, 64], F32, tag="h")
            nc.vector.tensor_scalar(out=h, in0=iota,
                                    scalar1=idxf,
                                    scalar2=ne2[:, t:t + 1],
                                    op0=OP.is_equal, op1=OP.mult)
            hBT = work.tile([64, 64], F32, tag="hBT")
            nc.vector.transpose(hBT, h)
            negmax = work.tile([64, 1], F32, tag="negmax")
            nc.vector.tensor_scalar(out=negmax, in0=m8[:, 0:1],
                                    scalar1=-1.0, scalar2=None, op0=OP.mult)
            e = work.tile([64, 64], F32, tag="e")
            nc.scalar.activation(out=e, in_=ps[:], func=ACT.Exp,
                                 bias=negmax, scale=1.0,
                                 accum_out=scoreb[:, t:t + 1])
            return hBT

        ps = pbw.tile([64, 64], F32, tag="bwps")
        bwd_prep(ps, T - 1, hist_tile(T - 1))
        hBT = bwd_dve(ps, T - 1)

        for t in range(T - 2, -1, -1):
            ps = pbw.tile([64, 64], F32, tag="bwps")
            last = bwd_prep(ps, t, hist_tile(t))
            h_mms(ps, hBT, after=last)
            hBT = bwd_dve(ps, t)

        # ---------------- epilogue ----------------
        tagsf = work.tile([64, T], F32, tag="tagsf")
        t8v = bass.AP(tensor=tags8.tensor, offset=tags8.offset,
                      ap=[tags8.ap[0], [8, T]])
        nc.vector.tensor_copy(tagsf, t8v)
        tagsm = work.tile([64, T], F32, tag="tagsm")
        nc.vector.tensor_tensor(out=tagsm, in0=tagsf, in1=maskt, op=OP.mult)
        tagsi = work.tile([64, T], mybir.dt.int32, tag="tagsi")
        nc.vector.tensor_copy(tagsi, tagsm)
        nc.sync.dma_start(tags_d, tagsi)
        recip = work.tile([64, T], F32, tag="recip")
        nc.vector.reciprocal(recip, scoreb)
        conf = work.tile([64, T], F32, tag="conf")
        nc.vector.tensor_tensor(out=conf, in0=recip, in1=maskt, op=OP.mult)
        nc.sync.dma_start(conf_d, conf)


def _get_compiled(T):
    key = ("nc", T)
    if key in _CACHE:
        return _CACHE[key]
    import concourse.bacc as bacc
    import concourse.tile as tile
    from concourse import mybir

    F32 = mybir.dt.float32
    U16 = mybir.dt.uint16
    I32 = mybir.dt.int32
    nc = bacc.Bacc("TRN2", target_bir_lowering=False, debug=False,
                   num_devices=_NCORES)

    ins_spec = [
        ("x2", [128, T, 32], F32),
        ("ne2", [64, T], F32),
        ("mask", [64, T], F32),
        ("trans_rep", [128, 32, 64], F32),
        ("S0", [128, 128], F32),
        ("S1", [128, 128], F32),
        ("tio_s", [64, 64], F32),
        ("tio_c", [64, 64], F32),
        ("iota", [64, 64], F32),
        ("trbB_own", [128, _CA + _CB, 32], F32),
        ("trbB_cross", [128, _CA + _CB, 32], F32),
        ("I128", [128, 128], F32),
        ("trans8", [2, _CP * 64], F32),
        ("ones2", [2, 128], F32),
    ]
    ins = tuple(
        nc.dram_tensor(name, shape, dt, kind="ExternalInput").ap()
        for name, shape, dt in ins_spec
    )
    outs = (
        nc.dram_tensor("tags", [64, T], I32, kind="ExternalOutput").ap(),
        nc.dram_tensor("conf", [64, T], F32, kind="ExternalOutput").ap(),
    )

    with tile.TileContext(nc) as tc:
        _build_tile_program(tc, outs, ins, T=T)
    nc.compile()
    _CACHE[key] = nc
    return nc


def _run(logits, transition_params, sequence_lengths, trace=False):
    from concourse.bass_utils import run_bass_kernel_spmd

    T = logits.shape[1]
    logits = np.asarray(logits, dtype=np.float32)
    trans = np.asarray(transition_params, dtype=np.float32)
    seq = np.asarray(sequence_lengths, dtype=np.int32)

    consts = _host_constants(trans)
    in_maps = []
    for c in range(_NCORES):
        sl = slice(c * _BL, (c + 1) * _BL)
        pc = _host_percore(logits[sl], seq[sl], T)
        m = {"x2": pc["x2"], "ne2": pc["ne2"], "mask": pc["mask"]}
        m.update(consts)
        in_maps.append(m)

    nc = _get_compiled(T)
    res = run_bass_kernel_spmd(nc, in_maps, list(range(_NCORES)),
                               trace=trace)
    tags = np.concatenate([np.asarray(res.results[c]["tags"])
                           for c in range(_NCORES)], axis=0)
    conf = np.concatenate([np.asarray(res.results[c]["conf"])
                           for c in range(_NCORES)], axis=0)
    return (tags.astype(np.int32), conf.astype(np.float32)), res


def kernel(logits, transition_params, sequence_lengths):
    (tags, conf), _ = _run(logits, transition_params, sequence_lengths)
    return tags, conf


# revision 11
# speedup vs baseline: 1.4099x; 1.0123x over previous
"""CRF Viterbi decode (nn_CRF, B=512 T=512 O=64) on 8 Trainium2 NeuronCores.

Pure data parallel: 64 sequences per core; the (64, 64) transition matrix and
derived constants are replicated.

Per-core layout: g = j_hi in {0,1}; partition p = g*64 + b; tag j = g*32+j_lo.

Forward (per step t): three engines build exact fp32 candidate blocks
concurrently; only DVE can max-reduce, so its reduce time is the floor:
  DVE:  hist = x2[:, t-1, :] + m2buf[:, t-1, :] (split-layout state); hX =
        cross-half partition swap of hist via two stream_shuffle ops with
        offset partition bases (HW-verified cross-quadrant moves); adds for
        columns 0..CA-1 in [own, cross] halves straight from hist/hX in
        SBUF; ALL segmented max-reduces -> m2buf[:, t, :].
  Pool: adds for columns CA..CA+CB-1 in per-partition [own, cross] i-order
        (max is order-invariant) reading hist/hX from SBUF (GPSIMD cannot
        touch PSUM and its TT ucode has no max).
  PE:   builds ts for the last CP=8 columns in one PSUM bank: a K=2
        group-indicator matmul lays down the trans block (the bank's one
        start=True), then identity-selector matmuls with broadcast rhs
        accumulate hist (own half) and hX (cross half).  Every element
        receives exactly one trans and one state contribution, so the PSUM
        accumulate is the same single fp32 add as the reference.

Backward, aligned in time t = T-1..0: cand_t = hist_t + trans[:, tag_{t+1}]
is built in PSUM from x2/m2 fold matmuls (pre-run, off the critical chain),
a Pool-computed hist window, and 4 one-hot h matmuls.  The argmax chain is
  max8 (top-8 values) -> max_index (first-index ties, = jnp.argmax)
  h = (iota == idx) * ne        (one tensor_scalar; ne=0 at t==L resets the
                                 chain so cand collapses to hist, reproducing
                                 init_tag/init_conf exactly)
  hBT = 32x32-blockwise DVE transpose, fixed up by straight/cross tables in
        the 4 K=32 h matmuls (PE quadrant layout as in the proven baseline).
max_index writes its 8 indices straight into tags8[:, t, :]; tags come from
tags8[:, :, 0] in the bulk epilogue.  Confidence = 1/sum exp(cand - max) via
ACT Exp with per-partition bias (bias = -max via a tiny tensor_scalar).

Hardware caveats kept from the previous session:
- matmul operands at partition base 64 crash the device (PE quadrant-3 bug)
  -- all contractions stay at base 0/32;
- start_tensor_calc=True lazily zeroes the whole per-partition 2KB PSUM
  region, so each accumulation group has exactly one start=True (a K=1
  zeroing matmul) and everything else accumulates.
"""
import numpy as np

_B, _T, _O = 512, 512, 64
_NCORES = 8
_BL = _B // _NCORES

_CA = 10           # DVE-adds tag columns (j_lo 0.._CA-1)
_CB = 14          # Pool-adds tag columns (j_lo _CA.._CA+_CB-1)
_CP = 8           # PE-built tag columns (j_lo _CA+_CB..31), one psum bank
_USE_MAXIDX = True  # max8/max_index argmax vs baseline 5-op argmax

_CACHE = {}


def _host_constants(trans):
    trans = np.ascontiguousarray(trans.astype(np.float32))
    transT = np.ascontiguousarray(trans.T)                  # [j, i]
    tr = transT.reshape(2, 32, 64)
    trans_rep = np.ascontiguousarray(
        np.broadcast_to(tr[:, None, :, :], (2, 64, 32, 64)).reshape(128, 32, 64)
    )
    S = np.zeros((2, 128, 128), np.float32)
    for h in range(2):
        for b in range(64):
            S[h, h * 64 + b, b] = 1.0
            S[h, h * 64 + b, 64 + b] = 1.0
    tio_s = np.ascontiguousarray(transT)                    # [64, 64]
    tio_c = np.ascontiguousarray(
        np.concatenate([tio_s[32:64], tio_s[0:32]], axis=0))
    iota = np.ascontiguousarray(
        np.broadcast_to(np.arange(64, dtype=np.float32), (64, 64)))
    # Pool B-column tables with per-partition [own-half, cross-half] i order:
    # trbB_own[p=(g,b), j_lo, i_lo] = trans[g*32+i_lo, g*32+(CA+j_lo)]
    # trbB_cross[p, j_lo, i_lo]     = trans[(1-g)*32+i_lo, g*32+(CA+j_lo)]
    CA, CB, CP = _CA, _CB, _CP
    g = (np.arange(128) // 64)[:, None, None]           # [128,1,1]
    jl = np.arange(CA + CB)[None, :, None]              # [1,CA+CB,1]
    il = np.arange(32)[None, None, :]                   # [1,1,32]
    trbB_own = np.ascontiguousarray(
        trans[g * 32 + il, g * 32 + jl].astype(np.float32))
    trbB_cross = np.ascontiguousarray(
        trans[(1 - g) * 32 + il, g * 32 + jl].astype(np.float32))
    I128 = np.eye(128, dtype=np.float32)
    # PE column block (j_lo = CA+CB..31): K=2 indicator rows select the
    # per-partition-group trans table; i-axis in [own, cross] order.
    gi = np.arange(2)[:, None, None]
    jp = (CA + CB + np.arange(CP))[None, :, None]
    trans8 = np.zeros((2, CP, 64), np.float32)
    trans8[:, :, 0:32] = trans[gi * 32 + il, gi * 32 + jp]
    trans8[:, :, 32:64] = trans[(1 - gi) * 32 + il, gi * 32 + jp]
    trans8 = np.ascontiguousarray(trans8.reshape(2, CP * 64))
    ones2 = np.zeros((2, 128), np.float32)
    ones2[0, 0:64] = 1.0
    ones2[1, 64:128] = 1.0
    return {
        "trans_rep": trans_rep,
        "S0": np.ascontiguousarray(S[0]),
        "S1": np.ascontiguousarray(S[1]),
        "tio_s": tio_s,
        "tio_c": tio_c,
        "iota": iota,
        "trbB_own": trbB_own,
        "trbB_cross": trbB_cross,
        "I128": I128,
        "trans8": trans8,
        "ones2": ones2,
    }


def _host_percore(logits_c, seq_c, T):
    x2 = np.ascontiguousarray(
        logits_c.astype(np.float32)
        .reshape(_BL, T, 2, 32).transpose(2, 0, 1, 3).reshape(128, T, 32)
    )
    # ne2[b, t] = 0 iff t == L_b: at backward step t == L the one-hot is
    # zeroed so cand_{L-1} collapses to hist_{L-1} (the reference's frozen
    # last_score).
    ne2 = np.ones((_BL, T), np.float32)
    sel = seq_c <= T - 1
    ne2[np.arange(_BL)[sel], seq_c[sel]] = 0.0
    mask = (np.arange(T)[None, :] < seq_c[:, None]).astype(np.float32)
    return {"x2": x2, "ne2": ne2, "mask": mask,
            "ne2u": ne2.astype(np.uint16)}


def _build_tile_program(tc, outs, ins, T):
    from contextlib import ExitStack
    import concourse.bass as bass
    from concourse import mybir
    from concourse.tile import add_dep_helper

    F32 = mybir.dt.float32
    U16 = mybir.dt.uint16
    AX = mybir.AxisListType
    OP = mybir.AluOpType
    ACT = mybir.ActivationFunctionType

    nc = tc.nc
    tags_d, conf_d = outs
    (x2_d, ne2_d, mask_d, transrep_d, s0_d, s1_d, tios_d, tioc_d,
     iota_d, trbo_d, trbc_d, i128_d, trans8_d, ones2_d) = ins

    CA, CB, CP = _CA, _CB, _CP

    def bcast_mid(ap2d, n):
        assert len(ap2d.ap) == 2, ap2d.ap
        return bass.AP(tensor=ap2d.tensor, offset=ap2d.offset,
                       ap=[ap2d.ap[0], [0, n], ap2d.ap[1]])

    def chain_mms(insts):
        for a, b in zip(insts[1:], insts[:-1]):
            add_dep_helper(a.ins, b.ins, sync=False,
                           reason="psum accumulation order")
        return insts[-1]

    with ExitStack() as ctx:
        consts = ctx.enter_context(tc.tile_pool(name="consts", bufs=1))
        big = ctx.enter_context(tc.tile_pool(name="big", bufs=1))
        work = ctx.enter_context(tc.tile_pool(name="work", bufs=2))
        tsA_p = ctx.enter_context(tc.tile_pool(name="tsA", bufs=2))
        tsB_p = ctx.enter_context(tc.tile_pool(name="tsB", bufs=2))
        histw = ctx.enter_context(tc.tile_pool(name="histw", bufs=4))
        ts8p = ctx.enter_context(
            tc.tile_pool(name="ts8p", bufs=2, space="PSUM"))
        pbw = ctx.enter_context(
            tc.tile_pool(name="pbw", bufs=3, space="PSUM"))
        hxp = ctx.enter_context(tc.tile_pool(name="hxp", bufs=2))

        trans_rep = consts.tile([128, 32, 64], F32)
        nc.sync.dma_start(trans_rep, transrep_d)
        S0 = consts.tile([128, 128], F32)
        nc.sync.dma_start(S0, s0_d)
        S1 = consts.tile([128, 128], F32)
        nc.sync.dma_start(S1, s1_d)
        tio_s = consts.tile([64, 64], F32)
        nc.sync.dma_start(tio_s, tios_d)
        tio_c = consts.tile([64, 64], F32)
        nc.sync.dma_start(tio_c, tioc_d)
        iota = consts.tile([64, 64], F32)
        nc.sync.dma_start(iota, iota_d)
        trbB_own = consts.tile([128, CA + CB, 32], F32)
        nc.sync.dma_start(trbB_own, trbo_d)
        trbB_cross = consts.tile([128, CA + CB, 32], F32)
        nc.sync.dma_start(trbB_cross, trbc_d)
        I128 = consts.tile([128, 128], F32)
        nc.sync.dma_start(I128, i128_d)
        trans8 = consts.tile([2, CP * 64], F32)
        nc.sync.dma_start(trans8, trans8_d)
        ones2 = consts.tile([2, 128], F32)
        nc.sync.dma_start(ones2, ones2_d)
        ne2 = consts.tile([64, T], F32)
        nc.sync.dma_start(ne2, ne2_d)
        maskt = consts.tile([64, T], F32)
        nc.sync.dma_start(maskt, mask_d)
        zl = consts.tile([1, 64], F32)
        nc.vector.memset(zl, 0.0)
        zl128 = consts.tile([1, 128], F32)
        nc.vector.memset(zl128, 0.0)
        zr128 = consts.tile([1, 128], F32)
        nc.vector.memset(zr128, 0.0)
        zr = consts.tile([1, 64], F32)
        nc.vector.memset(zr, 0.0)

        x2 = big.tile([128, T, 32], F32)
        NCH = 8
        CT = T // NCH
        for c in range(NCH):
            nc.sync.dma_start(x2[:, c * CT:(c + 1) * CT, :],
                              x2_d[:, c * CT:(c + 1) * CT, :])
        m2buf = big.tile([128, T, 32], F32)
        tags8 = big.tile([64, T, 8], U16)
        scoreb = big.tile([64, T], F32)

        # ---------------- forward ----------------
        # Division of labor (hardware constraints: GPSIMD cannot access
        # PSUM, and its software TensorTensor only implements add/mult —
        # no max): DVE reads the PSUM state directly and handles the adds
        # for columns 0..CA-1 plus ALL segmented max-reduces; Pool adds
        # columns CA..31 from an SBUF state copy made by the otherwise-idle
        # ACT engine.
        ident = list(range(32))
        for t in range(1, T):
            # DVE: previous state in split layout (own half per partition)
            # plus its cross-half partition swap, feeding Pool from SBUF.
            if t == 1:
                hist = x2[:, 0, :]
            else:
                histt = hxp.tile([128, 32], F32, tag="hist")
                nc.vector.tensor_tensor(out=histt, in0=x2[:, t - 1, :],
                                        in1=m2buf[:, t - 1, :], op=OP.add)
                hist = histt[:]
            hX = hxp.tile([128, 32], F32, tag="hX")
            nc.vector.stream_shuffle(hX[0:64, :], hist[64:128, :], ident)
            nc.vector.stream_shuffle(hX[64:128, :], hist[0:64, :], ident)
            def hist_b(n):
                return bass.AP(tensor=hist.tensor, offset=hist.offset,
                               ap=[hist.ap[0], [0, n], hist.ap[1]])

            # Pool: adds for columns CA..CA+CB-1 in [own, cross] i-order
            tsB = tsB_p.tile([128, CB, 64], F32, tag="tsB")
            nc.gpsimd.tensor_tensor(
                out=tsB[:, :, 0:32], in0=trbB_own[:, CA:CA + CB, :],
                in1=hist_b(CB), op=OP.add)
            nc.gpsimd.tensor_tensor(
                out=tsB[:, :, 32:64], in0=trbB_cross[:, CA:CA + CB, :],
                in1=bcast_mid(hX[:], CB), op=OP.add)
            # PE: builds ts for columns CA+CB..31 in PSUM: the constant
            # trans block (start=True, K=2 group indicator), then +hist and
            # +hX via identity selectors with broadcast rhs.  Each element
            # receives exactly one trans value and one state value, so the
            # PSUM accumulation is the same single fp32 add as the reference.
            ts8 = ts8p.tile([128, CP, 64], F32, tag="ts8")
            p0 = nc.tensor.matmul(
                bass.AP(tensor=ts8.tensor, offset=ts8.offset,
                        ap=[ts8.ap[0], [1, CP * 64]]),
                ones2, trans8, start=True, stop=False, skip_group_check=True)
            p1 = nc.tensor.matmul(
                ts8[:, :, 0:32], I128, hist_b(CP),
                start=False, stop=False, skip_group_check=True)
            p2 = nc.tensor.matmul(
                ts8[:, :, 32:64], I128, bcast_mid(hX[:], CP),
                start=False, stop=True, skip_group_check=True)
            chain_mms([p0, p1, p2])
            # DVE: adds for columns 0..CA-1 (own/cross halves from SBUF),
            # then the three segmented reduces
            tsA = tsA_p.tile([128, CA, 64], F32, tag="tsA")
            nc.vector.tensor_tensor(
                out=tsA[:, :, 0:32], in0=trbB_own[:, 0:CA, :],
                in1=hist_b(CA), op=OP.add)
            nc.vector.tensor_tensor(
                out=tsA[:, :, 32:64], in0=trbB_cross[:, 0:CA, :],
                in1=bcast_mid(hX[:], CA), op=OP.add)
            nc.vector.tensor_reduce(m2buf[:, t, 0:CA], tsA,
                                    axis=AX.X, op=OP.max)
            nc.vector.tensor_reduce(m2buf[:, t, CA + CB:32], ts8[:],
                                    axis=AX.X, op=OP.max)
            nc.vector.tensor_reduce(m2buf[:, t, CA:CA + CB], tsB,
                                    axis=AX.X, op=OP.max)

        # ---------------- backward ----------------
        def bwd_prep(ps, t, hb):
            """cand base: ps = hist_t replicated over i-columns."""
            i0 = nc.tensor.matmul(ps[:, :], zl, zr, start=True, stop=False,
                                  skip_group_check=True)
            i1 = nc.tensor.matmul(ps[:, 0:32], S0[:, 0:64], hb,
                                  start=False, stop=False,
                                  skip_group_check=True)
            i2 = nc.tensor.matmul(ps[:, 32:64], S1[:, 0:64], hb,
                                  start=False, stop=(t == T - 1),
                                  skip_group_check=True)
            return chain_mms([i0, i1, i2])

        def h_mms(ps, hBT, after):
            i1 = nc.tensor.matmul(ps[0:32, :], hBT[0:32, 0:32],
                                  tio_s[0:32, :], start=False, stop=False,
                                  skip_group_check=True)
            i2 = nc.tensor.matmul(ps[0:32, :], hBT[0:32, 32:64],
                                  tio_c[0:32, :], start=False, stop=False,
                                  skip_group_check=True)
            i3 = nc.tensor.matmul(ps[32:64, :], hBT[32:64, 0:32],
                                  tio_c[32:64, :], start=False, stop=False,
                                  skip_group_check=True)
            i4 = nc.tensor.matmul(ps[32:64, :], hBT[32:64, 32:64],
                                  tio_s[32:64, :], start=False, stop=True,
                                  skip_group_check=True)
            return chain_mms([after, i1, i2, i3, i4])

        def hist_tile(t):
            if t == 0:
                return x2[:, 0, :]  # init state: m2 only exists for t >= 1
            hb = histw.tile([128, 32], F32, tag="histw")
            nc.gpsimd.tensor_tensor(out=hb, in0=x2[:, t, :],
                                    in1=m2buf[:, t, :], op=OP.add)
            return hb

        def bwd_dve(ps, t):
            m8 = work.tile([64, 8], F32, tag="m8")
            nc.vector.max(m8, ps[:])
            nc.vector.max_index(tags8[:, t, :], m8, ps[:])
            idxf = work.tile([64, 1], F32, tag="idxf")
            nc.vector.tensor_copy(idxf, tags8[:, t, 0:1])
            h = work.tile([64, 64], F32, tag="h")
            nc.vector.tensor_scalar(out=h, in0=iota,
                                    scalar1=idxf,
                                    scalar2=ne2[:, t:t + 1],
                                    op0=OP.is_equal, op1=OP.mult)
            hBT = work.tile([64, 64], F32, tag="hBT")
            nc.vector.transpose(hBT, h)
            negmax = work.tile([64, 1], F32, tag="negmax")
            nc.vector.tensor_scalar(out=negmax, in0=m8[:, 0:1],
                                    scalar1=-1.0, scalar2=None, op0=OP.mult)
            e = work.tile([64, 64], F32, tag="e")
            nc.scalar.activation(out=e, in_=ps[:], func=ACT.Exp,
                                 bias=negmax, scale=1.0,
                                 accum_out=scoreb[:, t:t + 1])
            return hBT

        ps = pbw.tile([64, 64], F32, tag="bwps")
        bwd_prep(ps, T - 1, hist_tile(T - 1))
        hBT = bwd_dve(ps, T - 1)

        for t in range(T - 2, -1, -1):
            ps = pbw.tile([64, 64], F32, tag="bwps")
            last = bwd_prep(ps, t, hist_tile(t))
            h_mms(ps, hBT, after=last)
            hBT = bwd_dve(ps, t)

        # ---------------- epilogue ----------------
        tagsf = work.tile([64, T], F32, tag="tagsf")
        t8v = bass.AP(tensor=tags8.tensor, offset=tags8.offset,
                      ap=[tags8.ap[0], [8, T]])
        nc.vector.tensor_copy(tagsf, t8v)
        tagsm = work.tile([64, T], F32, tag="tagsm")
        nc.vector.tensor_tensor(out=tagsm, in0=tagsf, in1=maskt, op=OP.mult)
        tagsi = work.tile([64, T], mybir.dt.int32, tag="tagsi")
        nc.vector.tensor_copy(tagsi, tagsm)
        nc.sync.dma_start(tags_d, tagsi)
        recip = work.tile([64, T], F32, tag="recip")
        nc.vector.reciprocal(recip, scoreb)
        conf = work.tile([64, T], F32, tag="conf")
        nc.vector.tensor_tensor(out=conf, in0=recip, in1=maskt, op=OP.mult)
        nc.sync.dma_start(conf_d, conf)


def _get_compiled(T):
    key = ("nc", T)
    if key in _CACHE:
        return _CACHE[key]
    import concourse.bacc as bacc
    import concourse.tile as tile
    from concourse import mybir

    F32 = mybir.dt.float32
    U16 = mybir.dt.uint16
    I32 = mybir.dt.int32
    nc = bacc.Bacc("TRN2", target_bir_lowering=False, debug=False,
                   num_devices=_NCORES)

    ins_spec = [
        ("x2", [128, T, 32], F32),
        ("ne2", [64, T], F32),
        ("mask", [64, T], F32),
        ("trans_rep", [128, 32, 64], F32),
        ("S0", [128, 128], F32),
        ("S1", [128, 128], F32),
        ("tio_s", [64, 64], F32),
        ("tio_c", [64, 64], F32),
        ("iota", [64, 64], F32),
        ("trbB_own", [128, _CA + _CB, 32], F32),
        ("trbB_cross", [128, _CA + _CB, 32], F32),
        ("I128", [128, 128], F32),
        ("trans8", [2, _CP * 64], F32),
        ("ones2", [2, 128], F32),
    ]
    ins = tuple(
        nc.dram_tensor(name, shape, dt, kind="ExternalInput").ap()
        for name, shape, dt in ins_spec
    )
    outs = (
        nc.dram_tensor("tags", [64, T], I32, kind="ExternalOutput").ap(),
        nc.dram_tensor("conf", [64, T], F32, kind="ExternalOutput").ap(),
    )

    with tile.TileContext(nc) as tc:
        _build_tile_program(tc, outs, ins, T=T)
    nc.compile()
    _CACHE[key] = nc
    return nc


def _run(logits, transition_params, sequence_lengths, trace=False):
    from concourse.bass_utils import run_bass_kernel_spmd

    T = logits.shape[1]
    logits = np.asarray(logits, dtype=np.float32)
    trans = np.asarray(transition_params, dtype=np.float32)
    seq = np.asarray(sequence_lengths, dtype=np.int32)

    consts = _host_constants(trans)
    in_maps = []
    for c in range(_NCORES):
        sl = slice(c * _BL, (c + 1) * _BL)
        pc = _host_percore(logits[sl], seq[sl], T)
        m = {"x2": pc["x2"], "ne2": pc["ne2"], "mask": pc["mask"]}
        m.update(consts)
        in_maps.append(m)

    nc = _get_compiled(T)
    res = run_bass_kernel_spmd(nc, in_maps, list(range(_NCORES)),
                               trace=trace)
    tags = np.concatenate([np.asarray(res.results[c]["tags"])
                           for c in range(_NCORES)], axis=0)
    conf = np.concatenate([np.asarray(res.results[c]["conf"])
                           for c in range(_NCORES)], axis=0)
    return (tags.astype(np.int32), conf.astype(np.float32)), res


def kernel(logits, transition_params, sequence_lengths):
    (tags, conf), _ = _run(logits, transition_params, sequence_lengths)
    return tags, conf
